# revision 1
# baseline (speedup 1.0000x reference)
"""Trainium2 Bass kernel for windowed/global sparse attention (Swin-style
relative-position bias + 1 global token), data-parallel over batch on 8 cores.

Shapes: B=16, N=785 (1 global + 28x28 local), C=768, H=12 heads, d=64.

Per-core device program (2 batches/core, software-pipelined):
  - qT/kT computed transposed ([d, tokens]) so S^T = K @ Q^T needs no
    transposes anywhere; v computed natural ([tokens, d]) with a ones column
    appended per head so the P @ V matmul also yields softmax denominators.
  - softmax: exp(S + bias) = exp(S) * expB with expB = exp(bias) gathered on
    host from the (tiny) relative-position table at constant indices and
    shipped as a bf16 input; exp on ScalarE, multiply on VectorE (bf16 2x).
  - normalization: denominators from all 12 heads are staged to DRAM, one
    batched DVE reciprocal, then DMA-broadcast (0-step DRAM source) back to
    [128, N] and multiplied in place into O^T; proj consumes O^T directly as
    lhsT (again no transposes).
  - attention pairs run as a two-phase software pipeline: S/exp/multiply
    (exp-paced) for pair j overlaps a dense O-matmul convoy for pair j-1,
    and batch 1's QKV/V matmuls + batch 0's projection are emitted at lower
    priority as PE gap-filler for the attention phases — this keeps the
    TensorE activity monitor from re-throttling the PE clock to 1.2 GHz
    (the single biggest performance hazard found while profiling).
"""

import numpy as np
import ml_dtypes

import concourse.bass as bass
import concourse.bacc as bacc
import concourse.tile as tile
from concourse.tile import add_dep_helper
from concourse import mybir
from concourse.bass_utils import run_bass_kernel_spmd

F32 = mybir.dt.float32
BF16 = mybir.dt.bfloat16

WX = WY = 28
NGLO = 1
H = 12
L = WX * WY            # 784
N = NGLO + L           # 785
C = 768
HD = C // H            # 64
SCALE = HD ** -0.5
B = 16
N_CORES = 8
B_LOC = B // N_CORES   # 2
NCC = C // 128         # 6 contraction chunks
NKC = (N + 127) // 128  # 7 key/token chunks (last = 17 rows)
W = 786                # padded free width for N-sized tiles (even, 4B-aligned)

CG_N = [(0, 512), (512, 274)]
CG_C = [(0, 512), (512, 256)]


def _kr(kc):
    return min(128, N - kc * 128)


def build_nc():
    nc = bacc.Bacc(None, target_bir_lowering=False)

    xT_d = nc.dram_tensor("xT", [B_LOC, C, N], BF16, kind="ExternalInput")
    qkvwT_d = nc.dram_tensor("qkv_wT", [C, 3 * C], BF16, kind="ExternalInput")
    pwT_d = nc.dram_tensor("proj_wT", [C, C], BF16, kind="ExternalInput")
    pb_d = nc.dram_tensor("proj_b", [1, C], F32, kind="ExternalInput")
    expB_d = nc.dram_tensor("expB", [H, N, N], BF16, kind="ExternalInput")
    out_d = nc.dram_tensor("out", [B_LOC, N, C], F32, kind="ExternalOutput")
    dinv_d = nc.dram_tensor("dinv_scratch", [B_LOC, H, N], F32)

    with tile.TileContext(nc) as tc:
        with (
            tc.tile_pool(name="consts", bufs=1) as consts,
            tc.tile_pool(name="perb", bufs=2) as perb,
            tc.tile_pool(name="expbp", bufs=8) as expbp,
            tc.tile_pool(name="flow", bufs=4) as flow,
            tc.tile_pool(name="ptp", bufs=18) as ptp,
            tc.tile_pool(name="norm", bufs=1) as norm,
            tc.tile_pool(name="outp", bufs=2) as outp,
            tc.tile_pool(name="psum_s", bufs=4, space=bass.MemorySpace.PSUM) as psum_s,
        ):
            # ---- weights (resident, bf16); proj weights loaded last ----
            qkvw = []
            for cc in range(NCC):
                t = consts.tile([128, 3 * C], BF16, tag=f"qkvw{cc}", name=f"qkvw{cc}")
                qkvw.append(t)
            pw16 = []
            for cc in range(NCC):
                t = consts.tile([128, C], BF16, tag=f"pw{cc}", name=f"pw{cc}")
                pw16.append(t)
            pb_rep = consts.tile([128, C], F32, tag="pbrep")

            def emit_weight_loads_qkv():
                for cc in range(NCC):
                    nc.sync.dma_start(
                        qkvw[cc][:], qkvwT_d[cc * 128:(cc + 1) * 128, :]
                    )

            def emit_weight_loads_proj():
                for cc in range(NCC):
                    nc.sync.dma_start(
                        pw16[cc][:], pwT_d[cc * 128:(cc + 1) * 128, :]
                    )
                nc.sync.dma_start(pb_rep[:], pb_d[:].to_broadcast([128, C]))

            def emit_x(b):
                xts = []
                for cc in range(NCC):
                    t = perb.tile([128, W], BF16, tag=f"xt{cc}", name=f"xt{cc}_{b}")
                    nc.sync.dma_start(
                        t[:, 0:N], xT_d[b, cc * 128:(cc + 1) * 128, :]
                    )
                    nc.vector.memset(t[:, N:W], 0.0)
                    xts.append(t)
                return xts

            def emit_qkvT_chunk(b, xts, j, qT, kT, evac_vector):
                """produce qT[j] and kT[j] for batch b."""
                firsts = []
                for oc in (j, NCC + j):
                    ps = psum_s.tile([128, W], F32, tag="s", name=f"psqk{oc}_{b}")
                    for cc in range(NCC):
                        for (c0, cn) in CG_N:
                            mm = nc.tensor.matmul(
                                ps[:, c0:c0 + cn],
                                qkvw[cc][:, oc * 128:(oc + 1) * 128],
                                xts[cc][:, c0:c0 + cn],
                                start=(cc == 0),
                                stop=(cc == NCC - 1),
                            )
                            if cc == 0 and c0 == 0:
                                firsts.append(mm)
                    dst = qT[oc] if oc < NCC else kT[oc - NCC]
                    if evac_vector:
                        nc.vector.tensor_copy(dst[:, 0:N], ps[:, 0:N])
                    else:
                        nc.scalar.copy(dst[:, 0:N], ps[:, 0:N])
                    nc.vector.memset(dst[:, N:W], 0.0)
                return firsts

            def emit_v(b, xts):
                vp = [perb.tile([128, H * (HD + 1)], BF16, tag=f"vp{i}",
                                name=f"vp{i}_{b}") for i in range(NKC)]
                vfirsts = []
                for kc in range(NKC):
                    kr = _kr(kc)
                    ps = psum_s.tile([128, C], F32, tag="s", name=f"psv{kc}_{b}")
                    for cc in range(NCC):
                        for (c0, cn) in CG_C:
                            mm = nc.tensor.matmul(
                                ps[0:kr, c0:c0 + cn],
                                xts[cc][:, kc * 128:kc * 128 + kr],
                                qkvw[cc][:, 2 * C + c0:2 * C + c0 + cn],
                                start=(cc == 0),
                                stop=(cc == NCC - 1),
                            )
                            if cc == 0 and c0 == 0:
                                vfirsts.append(mm)
                    v3 = vp[kc][:].rearrange("p (h e) -> p h e", e=HD + 1)
                    nc.vector.tensor_copy(
                        v3[0:kr, :, 0:HD],
                        ps[0:kr, :].rearrange("p (h d) -> p h d", d=HD),
                    )
                    nc.vector.memset(v3[0:kr, :, HD:HD + 1], 1.0)
                return vp, vfirsts

            def alloc_oT(b):
                return [perb.tile([128, W], BF16, tag=f"oT{i}", name=f"oT{i}_{b}")
                        for i in range(NCC)]

            def emit_attn_pass1(b, j, qT, kT):
                """S + exp + expB-multiply for head pair (2j, 2j+1).
                Returns the pair's P tiles (fp8) and the pacer matmul."""
                pts = [[None, None] for _ in range(NKC)]
                pacer = None
                for kc in range(NKC):
                    kr = _kr(kc)
                    ps_ss = [
                        psum_s.tile([128, W], F32, tag="s",
                                    name=f"pss{2 * j + hh}_{kc}_{b}")
                        for hh in range(2)
                    ]
                    for (c0, cn) in CG_N:
                        for hh in range(2):
                            po = hh * 64
                            mm = nc.tensor.matmul(
                                ps_ss[hh][0:kr, c0:c0 + cn],
                                kT[j][po:po + 64, kc * 128:kc * 128 + kr],
                                qT[j][po:po + 64, c0:c0 + cn],
                                start=True,
                                stop=True,
                            )
                            if kc == 2 and pacer is None:
                                pacer = mm
                    for hh in range(2):
                        h = 2 * j + hh
                        ebt = expbp.tile([128, W], BF16, tag="expb",
                                         name=f"ebt{h}_{kc}_{b}")
                        nc.vector.memset(ebt[:, N:W], 0.0)
                        nc.sync.dma_start(
                            ebt[0:kr, 0:N],
                            expB_d[h, kc * 128:kc * 128 + kr, :],
                        )
                        es = flow.tile([128, W], BF16, tag="expS",
                                       name=f"es{h}_{kc}_{b}")
                        nc.scalar.activation(
                            es[0:kr, 0:W], ps_ss[hh][0:kr, 0:W],
                            mybir.ActivationFunctionType.Exp,
                        )
                        pt = ptp.tile([128, W], BF16, tag="pT",
                                      name=f"pt{h}_{kc}_{b}")
                        nc.vector.tensor_tensor(
                            pt[0:kr, 0:W],
                            es[0:kr, 0:W],
                            ebt[0:kr, 0:W],
                            mybir.AluOpType.mult,
                        )
                        pts[kc][hh] = pt
                return pts, pacer

            def emit_attn_pass2(b, j, pts, vp, oT, dall):
                """dense O-accumulation convoy for head pair (2j, 2j+1)."""
                for hh in range(2):
                    h = 2 * j + hh
                    ps_o = psum_s.tile([HD + 1, W], F32, tag="s",
                                       name=f"pso{h}_{b}")
                    for kc in range(NKC):
                        kr = _kr(kc)
                        for (c0, cn) in CG_N:
                            nc.tensor.matmul(
                                ps_o[:, c0:c0 + cn],
                                vp[kc][0:kr, h * (HD + 1):(h + 1) * (HD + 1)],
                                pts[kc][hh][0:kr, c0:c0 + cn],
                                start=(kc == 0),
                                stop=(kc == NKC - 1),
                            )
                    nc.vector.tensor_copy(
                        oT[j][hh * 64:hh * 64 + 64, 0:N], ps_o[0:64, 0:N]
                    )
                    dn = norm.tile([65, W], F32, tag="dn", bufs=1,
                                   name=f"dn{h}_{b}")
                    nc.vector.tensor_copy(dn[64:65, 0:N], ps_o[64:65, 0:N])
                    nc.sync.dma_start(dall[h:h + 1, 0:N], dn[64:65, 0:N])

            def emit_norm(b, oT, dall):

                # batched reciprocal + DMA broadcast + in-place normalize
                dinv = norm.tile([12, W], F32, tag="dinv", name=f"dinv_{b}")
                nc.vector.reciprocal(dinv[0:H, 0:N], dall[0:H, 0:N])
                nc.sync.dma_start(dinv_d[b], dinv[0:H, 0:N])
                for cc in range(NCC):
                    dr = norm.tile([128, W], F32, tag="drep", bufs=2,
                                   name=f"dr{cc}_{b}")
                    for hh in range(2):
                        row = dinv_d[b, 2 * cc + hh, :]
                        src = bass.AP(
                            tensor=row.tensor, offset=row.offset,
                            ap=[[0, 64]] + row.ap,
                        )
                        nc.sync.dma_start(dr[hh * 64:(hh + 1) * 64, 0:N], src)
                    nc.vector.tensor_tensor(
                        oT[cc][:, 0:N], oT[cc][:, 0:N], dr[:, 0:N],
                        mybir.AluOpType.mult,
                    )
                return oT

            def emit_proj(b, oT):
                pfirsts = []
                for tt in range(NKC):
                    ts_ = _kr(tt)
                    ps = psum_s.tile([128, C], F32, tag="s", name=f"psp{tt}_{b}")
                    for cc in range(NCC):
                        for (c0, cn) in CG_C:
                            mm = nc.tensor.matmul(
                                ps[0:ts_, c0:c0 + cn],
                                oT[cc][:, tt * 128:tt * 128 + ts_],
                                pw16[cc][:, c0:c0 + cn],
                                start=(cc == 0),
                                stop=(cc == NCC - 1),
                            )
                            if cc == 0 and c0 == 0:
                                pfirsts.append(mm)
                    ob = outp.tile([128, C], F32, tag="ob", name=f"ob{tt}_{b}")
                    nc.vector.tensor_tensor(
                        ob[0:ts_, :], ps[0:ts_, :], pb_rep[0:ts_, :],
                        mybir.AluOpType.add,
                    )
                    nc.sync.dma_start(
                        out_d[b, tt * 128:tt * 128 + ts_, :], ob[0:ts_, :]
                    )
                return pfirsts

            # software pipeline: batch 1's QKV work is emitted at lower
            # priority than batch 0's attention (and just-in-time between
            # batch 1's attention pairs) so the Tile scheduler uses it as PE
            # gap-filler — keeping the TensorE activity monitor from
            # re-throttling the clock during the attention phases.
            xts0 = emit_x(0)
            emit_weight_loads_qkv()
            qT0 = [perb.tile([128, W], BF16, tag=f"qT{i}", name=f"qT{i}_0")
                   for i in range(NCC)]
            kT0 = [perb.tile([128, W], BF16, tag=f"kT{i}", name=f"kT{i}_0")
                   for i in range(NCC)]
            for j in range(NCC):
                emit_qkvT_chunk(0, xts0, j, qT0, kT0, evac_vector=False)
            vp0, _ = emit_v(0, xts0)
            emit_weight_loads_proj()

            oT0 = alloc_oT(0)
            dall0 = norm.tile([12, W], F32, tag="dall", bufs=2, name="dall_0")
            pacers0 = []
            pend0 = []
            for j in range(NCC):
                if j >= 1:
                    emit_attn_pass2(0, j - 1, pend0[j - 1], vp0, oT0, dall0)
                pts_j, pac = emit_attn_pass1(0, j, qT0, kT0)
                pacers0.append(pac)
                pend0.append(pts_j)
            emit_attn_pass2(0, NCC - 1, pend0[NCC - 1], vp0, oT0, dall0)
            # fillers for batch-0 attention: x1 load + V1 + qkvT1, paced so
            # the greedy scheduler doesn't front-load them all at once
            xts1 = emit_x(1)
            vp1, vfirsts1 = emit_v(1, xts1)
            for kc, f in enumerate(vfirsts1):
                add_dep_helper(f.ins, pacers0[min(kc, NCC - 1)].ins, sync=False,
                               reason="pace v1 filler")
            emit_norm(0, oT0, dall0)

            qT1 = [perb.tile([128, W], BF16, tag=f"qT{i}", name=f"qT{i}_1")
                   for i in range(NCC)]
            kT1 = [perb.tile([128, W], BF16, tag=f"kT{i}", name=f"kT{i}_1")
                   for i in range(NCC)]
            oT1 = alloc_oT(1)
            dall1 = norm.tile([12, W], F32, tag="dall", bufs=2, name="dall_1")
            pacers1 = []
            pend1 = []
            for j in range(NCC):
                # just-in-time qkv chunk for pair j, used as gap-filler.
                # chunks 0-1 pace against late batch-0 attention; chunks 2-5
                # pace inside batch-1's attention (2-pair lead) so its
                # otherwise filler-starved phase gets PE work too.
                qf = emit_qkvT_chunk(1, xts1, j, qT1, kT1, evac_vector=True)
                pace = pacers0[j + 4] if j < 2 else pacers1[j - 2]
                for f in qf:
                    add_dep_helper(f.ins, pace.ins, sync=False,
                                   reason="pace qkvT1 filler")
                if j >= 1:
                    emit_attn_pass2(1, j - 1, pend1[j - 1], vp1, oT1, dall1)
                pts_j, pac = emit_attn_pass1(1, j, qT1, kT1)
                pacers1.append(pac)
                pend1.append(pts_j)
            emit_attn_pass2(1, NCC - 1, pend1[NCC - 1], vp1, oT1, dall1)
            # proj0: paced across batch-1 attention pairs as its PE filler
            pfirsts0 = emit_proj(0, oT0)
            for tt, f in enumerate(pfirsts0):
                add_dep_helper(f.ins, pacers1[min(tt, NCC - 1)].ins, sync=False,
                               reason="pace proj0 filler")
            emit_norm(1, oT1, dall1)
            emit_proj(1, oT1)

    nc.compile()
    return nc


def _relative_position_index():
    coords = np.stack(np.meshgrid(np.arange(WX), np.arange(WY), indexing="ij"))
    cf = coords.reshape(2, -1)
    rel = cf[:, :, None] - cf[:, None, :]
    rel = rel.transpose(1, 2, 0).astype(np.int64)
    rel[:, :, 0] += WX - 1
    rel[:, :, 1] += WY - 1
    rel[:, :, 0] *= 2 * WY - 1
    return rel.sum(-1)  # [L, L]


def _host_prep(x, qkv_w, proj_w, proj_b, rel_table, g2l, g2g):
    x = np.asarray(x, np.float32)
    qkv_w = np.asarray(qkv_w, np.float32)
    proj_w = np.asarray(proj_w, np.float32)
    proj_b = np.asarray(proj_b, np.float32)
    rel_table = np.asarray(rel_table, np.float32)
    g2l = np.asarray(g2l, np.float32)
    g2g = np.asarray(g2g, np.float32)

    bf16 = ml_dtypes.bfloat16
    xT = np.ascontiguousarray(x.transpose(0, 2, 1)).astype(bf16)   # [B, C, N]
    qkv_wT = np.ascontiguousarray(qkv_w.T).copy()                  # [C, 3C]
    qkv_wT[:, :C] *= SCALE                                         # fold q scale
    qkv_wT = qkv_wT.astype(bf16)
    proj_wT = np.ascontiguousarray(proj_w.T).astype(bf16)          # [C, C]
    pb = proj_b.reshape(1, C)

    # expB[h, k, q] = exp(bias[h, q, k]); exp applied at table granularity,
    # then expanded by the constant-index relative-position gather.
    ridx = _relative_position_index()
    et = np.exp(rel_table)                                         # [3025, H]
    eg2l = np.exp(g2l)                                             # [2, H, 1]
    eg2g = np.exp(g2g)                                             # [H, 1, 1]
    expB = np.empty((H, N, N), np.float32)
    expB[:, 1:, 1:] = et[ridx].transpose(2, 1, 0)                  # [H, k, q]
    expB[:, 0, 0] = eg2g[:, 0, 0]
    expB[:, 1:, 0] = eg2l[0][:, 0][None, :].T                      # global query
    expB[:, 0, 1:] = eg2l[1][:, 0][:, None]                        # global key
    expB16 = expB.astype(bf16)

    in_maps = []
    for i in range(N_CORES):
        in_maps.append({
            "xT": xT[i * B_LOC:(i + 1) * B_LOC],
            "qkv_wT": qkv_wT,
            "proj_wT": proj_wT,
            "proj_b": pb,
            "expB": expB16,
        })
    return in_maps


_NC = None


def get_nc():
    global _NC
    if _NC is None:
        _NC = build_nc()
    return _NC


def kernel(x, qkv_w, proj_w, proj_b, rel_table, g2l, g2g):
    in_maps = _host_prep(x, qkv_w, proj_w, proj_b, rel_table, g2l, g2g)
    nc = get_nc()
    res = run_bass_kernel_spmd(nc, in_maps, core_ids=list(range(N_CORES)))
    out = np.concatenate([res.results[i]["out"] for i in range(N_CORES)], axis=0)
    return out.astype(np.float32)



# revision 13
# speedup vs baseline: 1.0366x; 1.0366x over previous
"""Trainium2 Bass kernel for windowed/global sparse attention (Swin-style
relative-position bias + 1 global token), data-parallel over batch on 8 cores.

Shapes: B=16, N=785 (1 global + 28x28 local), C=768, H=12 heads, d=64.

Per-core device program (2 batches/core). Design notes:
  - qT/kT computed transposed ([d, tokens]) so S^T = K @ Q^T needs no
    transposes; v computed natural ([tokens, d]) with a ones column appended
    per head so the P @ V matmul also yields softmax denominators.
  - softmax: exp(S + bias) = exp(S) * expB with expB = exp(bias) gathered on
    host and shipped bf16; exp on ScalarE, multiply on VectorE (bf16 2x) with
    a fraction offloaded to GpSimd.
  - head-streamed schedule: for each (head h, key-chunk kc) step the TensorE
    stream carries S(h, kc) immediately followed by O(h-1, kc) — the dense
    O convoy rides inside the exp-paced S phase so the PE array never idles
    long enough for the HAM activity monitor to re-throttle the clock to
    1.2 GHz (43% of the old kernel's span ran cold).
  - PSUM discipline (8 banks): 2 rotating S slots + 1 O-convoy slot
    (allocated at first write) + 1 filler slot for qkv/v/proj convoys of the
    other/previous batch, which are spread between steps at (h, kc)
    granularity so no two convoys contend for the filler slot back-to-back.
  - denominators: O psum row 64 DMA'd straight to DRAM; reciprocal runs on
    a [128, 75]-reshaped view (0.5us instead of 5us at [12, 786]); 1/d is
    DMA-broadcast back (bf16) and multiplied into oT in place.
  - proj: bias applied via a ones-row matmul into the same psum accumulation,
    psum DMA'd straight to DRAM (no DVE add / evac).
"""

import numpy as np
import ml_dtypes

import concourse.bass as bass
import concourse.bacc as bacc
import concourse.tile as tile
from concourse import mybir
from concourse.bass_utils import run_bass_kernel_spmd

F32 = mybir.dt.float32
BF16 = mybir.dt.bfloat16

WX = WY = 28
NGLO = 1
H = 12
L = WX * WY            # 784
N = NGLO + L           # 785
C = 768
HD = C // H            # 64
SCALE = HD ** -0.5
B = 16
N_CORES = 8
B_LOC = B // N_CORES   # 2
NCC = C // 128         # 6 contraction chunks
NKC = (N + 127) // 128  # 7 key/token chunks (last = 17 rows)
W = 786                # padded free width for N-sized tiles (even)
DSTRIDE = 800          # flat stride for denominator rows in DRAM scratch
DPAD = 9600            # 12*800 = 128*75 for the reshaped reciprocal

CG_N = [(0, 512), (512, 274)]
CG_C = [(0, 512), (512, 256)]


def _kr(kc):
    return min(128, N - kc * 128)


def build_nc():
    nc = bacc.Bacc(None, target_bir_lowering=False)

    xT_d = nc.dram_tensor("xT", [B_LOC, C, N], BF16, kind="ExternalInput")
    qkvwT_d = nc.dram_tensor("qkv_wT", [C, 3 * C], BF16, kind="ExternalInput")
    pwT_d = nc.dram_tensor("proj_wT", [C, C], BF16, kind="ExternalInput")
    pb_d = nc.dram_tensor("proj_b", [1, C], BF16, kind="ExternalInput")
    expB_d = nc.dram_tensor("expB", [H, N, N], BF16, kind="ExternalInput")
    out_d = nc.dram_tensor("out", [B_LOC, N, C], F32, kind="ExternalOutput")
    dall_d = nc.dram_tensor("dall_scratch", [B_LOC, DPAD], F32)
    dinv_d = nc.dram_tensor("dinv_scratch", [B_LOC, DPAD], BF16)

    with tile.TileContext(nc) as tc:
        with (
            tc.tile_pool(name="consts", bufs=1) as consts,
            tc.tile_pool(name="perb", bufs=2) as perb,
            tc.tile_pool(name="expbp", bufs=8) as expbp,
            tc.tile_pool(name="flow", bufs=8) as flow,
            tc.tile_pool(name="ptp", bufs=16) as ptp,
            tc.tile_pool(name="norm", bufs=2) as normp,
            tc.tile_pool(name="outp", bufs=2) as outp,
            tc.tile_pool(name="psum_s", bufs=2, space=bass.MemorySpace.PSUM) as psum_s,
            tc.tile_pool(name="psum_o", bufs=1, space=bass.MemorySpace.PSUM) as psum_o,
            tc.tile_pool(name="psum_f", bufs=1, space=bass.MemorySpace.PSUM) as psum_f,
        ):
            # ---- resident weights ----
            qkvw = [consts.tile([128, 3 * C], BF16, tag=f"qkvw{cc}",
                                name=f"qkvw{cc}") for cc in range(NCC)]
            pw16 = [consts.tile([128, C], BF16, tag=f"pw{cc}", name=f"pw{cc}")
                    for cc in range(NCC)]
            pb16 = consts.tile([1, C], BF16, tag="pb16")
            ones = consts.tile([1, 128], BF16, tag="ones")
            nc.vector.memset(ones[:], 1.0)

            def load_qkvw():
                for cc in range(NCC):
                    nc.sync.dma_start(
                        qkvw[cc][:], qkvwT_d[cc * 128:(cc + 1) * 128, :])

            def load_pw():
                for cc in range(NCC):
                    nc.sync.dma_start(
                        pw16[cc][:], pwT_d[cc * 128:(cc + 1) * 128, :])
                nc.sync.dma_start(pb16[:], pb_d[:])

            def load_x(b):
                xts = []
                for cc in range(NCC):
                    t = perb.tile([128, W], BF16, tag=f"xt{cc}",
                                  name=f"xt{cc}_{b}")
                    nc.sync.dma_start(
                        t[:, 0:N], xT_d[b, cc * 128:(cc + 1) * 128, :])
                    xts.append(t)
                return xts

            def alloc_qkT(b):
                qT = [perb.tile([128, W], BF16, tag=f"qT{i}", name=f"qT{i}_{b}")
                      for i in range(NCC)]
                kT = [perb.tile([128, W], BF16, tag=f"kT{i}", name=f"kT{i}_{b}")
                      for i in range(NCC)]
                return qT, kT

            def emit_qkT_convoy(b, oc, xts, qT, kT, pool, ptag):
                """one output chunk (128 cols of q or k), contraction over C."""
                ps = pool.tile([128, W], F32, tag=ptag, name=f"psqk{oc}_{b}")
                for cc in range(NCC):
                    for (c0, cn) in CG_N:
                        nc.tensor.matmul(
                            ps[:, c0:c0 + cn],
                            qkvw[cc][:, oc * 128:(oc + 1) * 128],
                            xts[cc][:, c0:c0 + cn],
                            start=(cc == 0), stop=(cc == NCC - 1),
                        )
                dst = qT[oc] if oc < NCC else kT[oc - NCC]
                nc.scalar.copy(dst[:, 0:N], ps[:, 0:N])

            def alloc_vp(b):
                return [perb.tile([128, H * (HD + 1)], BF16, tag=f"vp{i}",
                                  name=f"vp{i}_{b}") for i in range(NKC)]

            def emit_v_convoy(b, kc, xts, vp, pool, ptag):
                kr = _kr(kc)
                ps = pool.tile([128, W], F32, tag=ptag, name=f"psv{kc}_{b}")
                for cc in range(NCC):
                    for (c0, cn) in CG_C:
                        nc.tensor.matmul(
                            ps[0:kr, c0:c0 + cn],
                            xts[cc][:, kc * 128:kc * 128 + kr],
                            qkvw[cc][:, 2 * C + c0:2 * C + c0 + cn],
                            start=(cc == 0), stop=(cc == NCC - 1),
                        )
                v3 = vp[kc][:].rearrange("p (h e) -> p h e", e=HD + 1)
                nc.vector.tensor_copy(
                    v3[0:kr, :, 0:HD],
                    ps[0:kr, 0:C].rearrange("p (h d) -> p h d", d=HD),
                )
                nc.vector.memset(v3[0:kr, :, HD:HD + 1], 1.0)

            def emit_proj_convoy(b, tt, oT, pool, ptag):
                """one token chunk of the projection, bias via ones-matmul."""
                ts_ = _kr(tt)
                ps = pool.tile([128, W], F32, tag=ptag, name=f"psp{tt}_{b}")
                for (c0, cn) in CG_C:
                    nc.tensor.matmul(
                        ps[0:ts_, c0:c0 + cn],
                        ones[0:1, 0:ts_],
                        pb16[0:1, c0:c0 + cn],
                        start=True, stop=False,
                    )
                    for cc in range(NCC):
                        nc.tensor.matmul(
                            ps[0:ts_, c0:c0 + cn],
                            oT[cc][:, tt * 128:tt * 128 + ts_],
                            pw16[cc][:, c0:c0 + cn],
                            start=False, stop=(cc == NCC - 1),
                        )
                ob = outp.tile([128, C], F32, tag="ob", name=f"ob{tt}_{b}")
                nc.vector.tensor_copy(ob[0:ts_, :], ps[0:ts_, 0:C])
                nc.sync.dma_start(
                    out_d[b, tt * 128:tt * 128 + ts_, :], ob[0:ts_, :])

            def emit_norm_recip(b):
                da = normp.tile([128, 75], F32, tag="da", name=f"da_{b}")
                nc.sync.dma_start(
                    da[:], dall_d[b].rearrange("(p f) -> p f", f=75))
                di = normp.tile([128, 75], BF16, tag="di", name=f"di_{b}")
                with nc.allow_low_precision(reason="1/d broadcast in bf16"):
                    nc.vector.reciprocal(di[:], da[:])
                nc.sync.dma_start(
                    dinv_d[b].rearrange("(p f) -> p f", f=75), di[:])

            def emit_norm_pair(b, j, oT):
                dr = normp.tile([128, W], BF16, tag="drep", name=f"dr{j}_{b}")
                for hh in range(2):
                    row = dinv_d[b, (2 * j + hh) * DSTRIDE:
                                 (2 * j + hh) * DSTRIDE + N]
                    src = bass.AP(tensor=row.tensor, offset=row.offset,
                                  ap=[[0, 64]] + row.ap)
                    nc.sync.dma_start(dr[hh * 64:(hh + 1) * 64, 0:N], src)
                nc.vector.tensor_tensor(
                    oT[j][:, 0:N], oT[j][:, 0:N], dr[:, 0:N],
                    mybir.AluOpType.mult)

            # ---------------- attention ----------------
            def emit_attention(b, qT, kT, vp, oT, fillers):
                """head-streamed: per (h, kc) step the PE stream carries
                S(h, kc) then O(h-1, kc); filler closures attached to (h, kc)
                run after that step's emission. fillers[(h, kc)] -> [fn]."""
                steps = [(h, kc) for h in range(H) for kc in range(NKC)]
                pts = {}
                psO = {}

                def issue_ebt(idx):
                    h, kc = steps[idx]
                    kr = _kr(kc)
                    t = expbp.tile([128, W], BF16, tag="expb",
                                   name=f"ebt{h}_{kc}_{b}")
                    eng = nc.gpsimd if (idx % 3 == 2) else nc.sync
                    eng.dma_start(t[0:kr, 0:N],
                                  expB_d[h, kc * 128:kc * 128 + kr, :])
                    return t

                ebt_q = {}
                for i in range(4):
                    ebt_q[i] = issue_ebt(i)

                def emit_O_step(h, kc):
                    kr = _kr(kc)
                    if kc == 0:
                        psO[h] = psum_o.tile([HD + 1, W], F32, tag="o",
                                             name=f"pso{h}_{b}")
                    pt = pts.pop((h, kc))
                    for (c0, cn) in CG_N:
                        nc.tensor.matmul(
                            psO[h][:, c0:c0 + cn],
                            vp[kc][0:kr, h * (HD + 1):(h + 1) * (HD + 1)],
                            pt[0:kr, c0:c0 + cn],
                            start=(kc == 0), stop=(kc == NKC - 1),
                        )

                def emit_O_evac(h):
                    j, hh = h // 2, h % 2
                    nc.vector.tensor_copy(
                        oT[j][hh * 64:hh * 64 + 64, 0:N], psO[h][0:64, 0:N])
                    dn = normp.tile([65, W], F32, tag="dn", name=f"dn{h}_{b}")
                    nc.vector.tensor_copy(dn[64:65, 0:W], psO[h][64:65, 0:W])
                    nc.sync.dma_start(
                        dall_d[b, h * DSTRIDE:h * DSTRIDE + W],
                        dn[64:65, 0:W])

                for i, (h, kc) in enumerate(steps):
                    j = h // 2
                    po = (h % 2) * 64
                    kr = _kr(kc)
                    # S matmuls
                    ps = psum_s.tile([128, W], F32, tag="s",
                                     name=f"pss{h}_{kc}_{b}")
                    for (c0, cn) in CG_N:
                        nc.tensor.matmul(
                            ps[0:kr, c0:c0 + cn],
                            kT[j][po:po + 64, kc * 128:kc * 128 + kr],
                            qT[j][po:po + 64, c0:c0 + cn],
                            start=True, stop=True,
                        )
                    # O for previous head rides in the same step
                    if h > 0:
                        emit_O_step(h - 1, kc)
                    # exp + expB multiply
                    es = flow.tile([128, W], BF16, tag="expS",
                                   name=f"es{h}_{kc}_{b}")
                    nc.scalar.activation(
                        es[0:kr, 0:W], ps[0:kr, 0:W],
                        mybir.ActivationFunctionType.Exp)
                    pt = ptp.tile([128, W], BF16, tag="pT",
                                  name=f"pt{h}_{kc}_{b}")
                    meng = nc.gpsimd if (i % 6 == 5) else nc.vector
                    meng.tensor_tensor(
                        pt[0:kr, 0:N], es[0:kr, 0:N], ebt_q.pop(i)[0:kr, 0:N],
                        mybir.AluOpType.mult)
                    pts[(h, kc)] = pt
                    if i + 4 < len(steps):
                        ebt_q[i + 4] = issue_ebt(i + 4)
                    # previous head's O evac at its boundary
                    if kc == NKC - 1 and h > 0:
                        emit_O_evac(h - 1)
                    for f in fillers.get((h, kc), []):
                        f()
                # trailing O convoy for the last head
                for kc in range(NKC):
                    emit_O_step(H - 1, kc)
                emit_O_evac(H - 1)
                for f in fillers.get((H, 0), []):
                    f()

            # ---------------- program ----------------
            xts0 = load_x(0)
            load_qkvw()
            qT0, kT0 = alloc_qkT(0)
            qT1, kT1 = alloc_qkT(1)
            # head phase: q0, k0, q1 + V0 kc0-5 (S-slot rotation, pre-attn)
            emit_qkT_convoy(0, 0, xts0, qT0, kT0, psum_s, "s")
            emit_qkT_convoy(0, NCC + 0, xts0, qT0, kT0, psum_s, "s")
            emit_qkT_convoy(0, 1, xts0, qT0, kT0, psum_s, "s")
            vp0 = alloc_vp(0)
            for kc in range(6):
                emit_v_convoy(0, kc, xts0, vp0, psum_s, "s")
            oT0 = [perb.tile([128, W], BF16, tag=f"oT{i}", name=f"oT{i}_0")
                   for i in range(NCC)]
            oT1 = [perb.tile([128, W], BF16, tag=f"oT{i}", name=f"oT{i}_1")
                   for i in range(NCC)]
            vp1 = alloc_vp(1)
            xts1_box = {}

            def qk0(oc):
                return lambda: emit_qkT_convoy(0, oc, xts0, qT0, kT0,
                                               psum_f, "f")

            def qk1(oc):
                return lambda: emit_qkT_convoy(1, oc, xts1_box[0], qT1, kT1,
                                               psum_f, "f")

            def v0(kc):
                return lambda: emit_v_convoy(0, kc, xts0, vp0, psum_f, "f")

            def v1(kc):
                return lambda: emit_v_convoy(1, kc, xts1_box[0], vp1,
                                             psum_f, "f")

            def load_x1():
                xts1_box[0] = load_x(1)

            KOF = NCC  # k output-chunk offset
            fill0 = {
                (0, 2): [v0(6)],
                (0, 3): [load_x1],
                (0, 4): [qk0(KOF + 1)],          # k1
                (0, 6): [qk0(2)],                # q2
                (1, 2): [qk0(KOF + 2)],          # k2
                (1, 5): [load_pw],
                (1, 6): [qk0(3)],                # q3
                (2, 2): [qk0(KOF + 3)],          # k3
                (2, 6): [qk0(4)],                # q4
                (3, 2): [qk0(KOF + 4)],          # k4
                (3, 6): [qk0(5)],                # q5
                (4, 2): [qk0(KOF + 5)],          # k5
                (5, 2): [qk1(0)],
                (5, 6): [qk1(KOF + 0)],
                (6, 2): [qk1(1)],
                (6, 6): [qk1(KOF + 1)],
                (7, 2): [qk1(2)],
                (7, 6): [qk1(KOF + 2)],
                (8, 2): [qk1(3)],
                (8, 6): [qk1(KOF + 3)],
                (9, 2): [qk1(4)],
                (9, 6): [qk1(KOF + 4)],
                (10, 2): [qk1(5)],
                (10, 6): [qk1(KOF + 5)],
                (11, 2): [v1(0)],
                (11, 4): [v1(1)],
                (11, 6): [v1(2)],
                (H, 0): [v1(3)],
            }

            def proj0(tt):
                return lambda: emit_proj_convoy(0, tt, oT0, psum_f, "f")

            fill1 = {
                (0, 1): [v1(4)],
                (0, 3): [v1(5)],
                (0, 5): [v1(6)],
                (1, 1): [lambda: emit_norm_recip(0)],
                (1, 3): [lambda: emit_norm_pair(0, 0, oT0)],
                (1, 5): [lambda: emit_norm_pair(0, 1, oT0)],
                (2, 1): [lambda: emit_norm_pair(0, 2, oT0)],
                (2, 3): [lambda: emit_norm_pair(0, 3, oT0)],
                (2, 5): [lambda: emit_norm_pair(0, 4, oT0)],
                (3, 1): [lambda: emit_norm_pair(0, 5, oT0)],
                (3, 5): [proj0(0)],
                (4, 2): [proj0(1)],
                (4, 6): [proj0(2)],
                (5, 3): [proj0(3)],
                (5, 6): [proj0(4)],
                (6, 3): [proj0(5)],
                (6, 6): [proj0(6)],
            }

            emit_attention(0, qT0, kT0, vp0, oT0, fill0)
            emit_attention(1, qT1, kT1, vp1, oT1, fill1)

            # tail: batch 1 normalize + projection (alternate psum tags so
            # consecutive convoys don't serialize on one slot)
            emit_norm_recip(1)
            for j in range(NCC):
                emit_norm_pair(1, j, oT1)
            for tt in range(NKC):
                if tt % 2 == 0:
                    emit_proj_convoy(1, tt, oT1, psum_s, "s")
                else:
                    emit_proj_convoy(1, tt, oT1, psum_f, "f")

    nc.compile()
    return nc


def _relative_position_index():
    coords = np.stack(np.meshgrid(np.arange(WX), np.arange(WY), indexing="ij"))
    cf = coords.reshape(2, -1)
    rel = cf[:, :, None] - cf[:, None, :]
    rel = rel.transpose(1, 2, 0).astype(np.int64)
    rel[:, :, 0] += WX - 1
    rel[:, :, 1] += WY - 1
    rel[:, :, 0] *= 2 * WY - 1
    return rel.sum(-1)  # [L, L]


def _host_prep(x, qkv_w, proj_w, proj_b, rel_table, g2l, g2g):
    x = np.asarray(x, np.float32)
    qkv_w = np.asarray(qkv_w, np.float32)
    proj_w = np.asarray(proj_w, np.float32)
    proj_b = np.asarray(proj_b, np.float32)
    rel_table = np.asarray(rel_table, np.float32)
    g2l = np.asarray(g2l, np.float32)
    g2g = np.asarray(g2g, np.float32)

    bf16 = ml_dtypes.bfloat16
    xT = np.ascontiguousarray(x.transpose(0, 2, 1)).astype(bf16)   # [B, C, N]
    qkv_wT = np.ascontiguousarray(qkv_w.T).copy()                  # [C, 3C]
    qkv_wT[:, :C] *= SCALE                                         # fold q scale
    qkv_wT = qkv_wT.astype(bf16)
    proj_wT = np.ascontiguousarray(proj_w.T).astype(bf16)          # [C, C]
    pb = proj_b.reshape(1, C).astype(bf16)

    # expB[h, k, q] = exp(bias[h, q, k]); exp applied at table granularity,
    # then expanded by the constant-index relative-position gather.
    ridx = _relative_position_index()
    et = np.exp(rel_table)                                         # [3025, H]
    eg2l = np.exp(g2l)                                             # [2, H, 1]
    eg2g = np.exp(g2g)                                             # [H, 1, 1]
    expB = np.empty((H, N, N), np.float32)
    expB[:, 1:, 1:] = et[ridx].transpose(2, 1, 0)                  # [H, k, q]
    expB[:, 0, 0] = eg2g[:, 0, 0]
    expB[:, 1:, 0] = eg2l[0][:, 0][None, :].T                      # global query
    expB[:, 0, 1:] = eg2l[1][:, 0][:, None]                        # global key
    expB16 = expB.astype(bf16)

    in_maps = []
    for i in range(N_CORES):
        in_maps.append({
            "xT": xT[i * B_LOC:(i + 1) * B_LOC],
            "qkv_wT": qkv_wT,
            "proj_wT": proj_wT,
            "proj_b": pb,
            "expB": expB16,
        })
    return in_maps


_NC = None


def get_nc():
    global _NC
    if _NC is None:
        _NC = build_nc()
    return _NC


def kernel(x, qkv_w, proj_w, proj_b, rel_table, g2l, g2g):
    in_maps = _host_prep(x, qkv_w, proj_w, proj_b, rel_table, g2l, g2g)
    nc = get_nc()
    res = run_bass_kernel_spmd(nc, in_maps, core_ids=list(range(N_CORES)))
    out = np.concatenate([res.results[i]["out"] for i in range(N_CORES)], axis=0)
    return out.astype(np.float32)


# revision 22
# speedup vs baseline: 1.1296x; 1.0898x over previous
"""Trainium2 Bass kernel for windowed/global sparse attention (Swin-style
relative-position bias + 1 global token), data-parallel over batch on 8 cores.

Shapes: B=16, N=785 (1 global + 28x28 local), C=768, H=12 heads, d=64.

Per-core device program (2 batches/core). Design notes:
  - qT/kT computed transposed ([d, tokens]) so S^T = K @ Q^T needs no
    transposes; v computed natural ([tokens, d]) with a ones column appended
    per head so the P @ V matmul also yields softmax denominators.
  - softmax: exp(S + bias) = exp(S) * expB with expB = exp(bias) gathered on
    host and shipped bf16; exp on ScalarE, multiply on VectorE (bf16 2x) with
    a fraction offloaded to GpSimd.
  - head-streamed schedule: for each (head h, key-chunk kc) step the TensorE
    stream carries S(h, kc) immediately followed by O(h-1, kc) — the dense
    O convoy rides inside the exp-paced S phase so the PE array never idles
    long enough for the HAM activity monitor to re-throttle the clock to
    1.2 GHz (43% of the old kernel's span ran cold).
  - PSUM discipline (8 banks): 2 rotating S slots + 1 O-convoy slot
    (allocated at first write) + 1 filler slot for qkv/v/proj convoys of the
    other/previous batch, which are spread between steps at (h, kc)
    granularity so no two convoys contend for the filler slot back-to-back.
  - denominators: O psum row 64 DMA'd straight to DRAM; reciprocal runs on
    a [128, 75]-reshaped view (0.5us instead of 5us at [12, 786]); 1/d is
    DMA-broadcast back (bf16) and multiplied into oT in place.
  - proj: bias applied via a ones-row matmul into the same psum accumulation,
    psum DMA'd straight to DRAM (no DVE add / evac).
"""

import numpy as np
import ml_dtypes

import concourse.bass as bass
import concourse.bacc as bacc
import concourse.tile as tile
from concourse import mybir
from concourse.bass_utils import run_bass_kernel_spmd

F32 = mybir.dt.float32
BF16 = mybir.dt.bfloat16

WX = WY = 28
NGLO = 1
H = 12
L = WX * WY            # 784
N = NGLO + L           # 785
C = 768
HD = C // H            # 64
SCALE = HD ** -0.5
B = 16
N_CORES = 8
B_LOC = B // N_CORES   # 2
NCC = C // 128         # 6 contraction chunks
NKC = (N + 127) // 128  # 7 key/token chunks (last = 17 rows)
W = 786                # padded free width for N-sized tiles (even)
DSTRIDE = 800          # flat stride for denominator rows in DRAM scratch
DPAD = 9600            # 12*800 = 128*75 for the reshaped reciprocal

CG_N = [(0, 512), (512, 274)]
CG_C = [(0, 512), (512, 256)]


def _kr(kc):
    return min(128, N - kc * 128)


def build_nc():
    nc = bacc.Bacc(None, target_bir_lowering=False)

    xT_d = nc.dram_tensor("xT", [B_LOC, C, N], BF16, kind="ExternalInput")
    qkvwT_d = nc.dram_tensor("qkv_wT", [C, 3 * C], BF16, kind="ExternalInput")
    pwT_d = nc.dram_tensor("proj_wT", [C, C], BF16, kind="ExternalInput")
    pb_d = nc.dram_tensor("proj_b", [1, C], BF16, kind="ExternalInput")
    expB_d = nc.dram_tensor("expB", [H, N, N], BF16, kind="ExternalInput")
    out_d = nc.dram_tensor("out", [B_LOC, N, C], F32, kind="ExternalOutput")
    dall_d = nc.dram_tensor("dall_scratch", [B_LOC, DPAD], F32)
    dinv_d = nc.dram_tensor("dinv_scratch", [B_LOC, DPAD], BF16)

    with tile.TileContext(nc) as tc:
        with (
            tc.tile_pool(name="consts", bufs=1) as consts,
            tc.tile_pool(name="perb", bufs=2) as perb,
            tc.tile_pool(name="expbp", bufs=8) as expbp,
            tc.tile_pool(name="flow", bufs=8) as flow,
            tc.tile_pool(name="ptp", bufs=16) as ptp,
            tc.tile_pool(name="norm", bufs=2) as normp,
            tc.tile_pool(name="outp", bufs=2) as outp,
            tc.tile_pool(name="psum_s", bufs=2, space=bass.MemorySpace.PSUM) as psum_s,
            tc.tile_pool(name="psum_o", bufs=1, space=bass.MemorySpace.PSUM) as psum_o,
            tc.tile_pool(name="psum_f", bufs=1, space=bass.MemorySpace.PSUM) as psum_f,
        ):
            # ---- resident weights ----
            qkvw = [consts.tile([128, 3 * C], BF16, tag=f"qkvw{cc}",
                                name=f"qkvw{cc}") for cc in range(NCC)]
            pw16 = [consts.tile([128, C], BF16, tag=f"pw{cc}", name=f"pw{cc}")
                    for cc in range(NCC)]
            pb16 = consts.tile([1, C], BF16, tag="pb16")
            ones = consts.tile([1, 128], BF16, tag="ones")
            nc.vector.memset(ones[:], 1.0)

            def load_qkvw():
                for cc in range(NCC):
                    nc.gpsimd.dma_start(
                        qkvw[cc][:], qkvwT_d[cc * 128:(cc + 1) * 128, :])

            def load_pw():
                for cc in range(NCC):
                    nc.sync.dma_start(
                        pw16[cc][:], pwT_d[cc * 128:(cc + 1) * 128, :])
                nc.sync.dma_start(pb16[:], pb_d[:])

            def load_x(b):
                xts = []
                for cc in range(NCC):
                    t = perb.tile([128, W], BF16, tag=f"xt{cc}",
                                  name=f"xt{cc}_{b}")
                    nc.sync.dma_start(
                        t[:, 0:N], xT_d[b, cc * 128:(cc + 1) * 128, :])
                    xts.append(t)
                return xts

            def alloc_qkT(b):
                qT = [perb.tile([128, W], BF16, tag=f"qT{i}", name=f"qT{i}_{b}")
                      for i in range(NCC)]
                kT = [perb.tile([128, W], BF16, tag=f"kT{i}", name=f"kT{i}_{b}")
                      for i in range(NCC)]
                return qT, kT

            def emit_qkT_convoy(b, oc, xts, qT, kT, pool, ptag):
                """one output chunk (128 cols of q or k), contraction over C."""
                ps = pool.tile([128, W], F32, tag=ptag, name=f"psqk{oc}_{b}")
                for cc in range(NCC):
                    for (c0, cn) in CG_N:
                        nc.tensor.matmul(
                            ps[:, c0:c0 + cn],
                            qkvw[cc][:, oc * 128:(oc + 1) * 128],
                            xts[cc][:, c0:c0 + cn],
                            start=(cc == 0), stop=(cc == NCC - 1),
                        )
                dst = qT[oc] if oc < NCC else kT[oc - NCC]
                nc.scalar.copy(dst[:, 0:N], ps[:, 0:N])

            def alloc_vp(b):
                return [perb.tile([128, H * (HD + 1)], BF16, tag=f"vp{i}",
                                  name=f"vp{i}_{b}") for i in range(NKC)]

            def emit_v_convoy(b, kc, xts, vp, pool, ptag):
                kr = _kr(kc)
                ps = pool.tile([128, W], F32, tag=ptag, name=f"psv{kc}_{b}")
                for cc in range(NCC):
                    for (c0, cn) in CG_C:
                        nc.tensor.matmul(
                            ps[0:kr, c0:c0 + cn],
                            xts[cc][:, kc * 128:kc * 128 + kr],
                            qkvw[cc][:, 2 * C + c0:2 * C + c0 + cn],
                            start=(cc == 0), stop=(cc == NCC - 1),
                        )
                v3 = vp[kc][:].rearrange("p (h e) -> p h e", e=HD + 1)
                nc.vector.tensor_copy(
                    v3[0:kr, :, 0:HD],
                    ps[0:kr, 0:C].rearrange("p (h d) -> p h d", d=HD),
                )
                nc.vector.memset(v3[0:kr, :, HD:HD + 1], 1.0)

            def emit_proj_convoy(b, tt, oT, pool, ptag):
                """one token chunk of the projection, bias via ones-matmul."""
                ts_ = _kr(tt)
                ps = pool.tile([128, W], F32, tag=ptag, name=f"psp{tt}_{b}")
                for (c0, cn) in CG_C:
                    nc.tensor.matmul(
                        ps[0:ts_, c0:c0 + cn],
                        ones[0:1, 0:ts_],
                        pb16[0:1, c0:c0 + cn],
                        start=True, stop=False,
                    )
                    for cc in range(NCC):
                        nc.tensor.matmul(
                            ps[0:ts_, c0:c0 + cn],
                            oT[cc][:, tt * 128:tt * 128 + ts_],
                            pw16[cc][:, c0:c0 + cn],
                            start=False, stop=(cc == NCC - 1),
                        )
                ob = outp.tile([128, C], F32, tag="ob", name=f"ob{tt}_{b}")
                nc.vector.tensor_copy(ob[0:ts_, :], ps[0:ts_, 0:C])
                nc.sync.dma_start(
                    out_d[b, tt * 128:tt * 128 + ts_, :], ob[0:ts_, :])

            def emit_norm_pair(b, j, oT):
                """in-place oT[j] *= 1/d: per-pair reciprocal on a [64, 25]
                reshaped view of the pair's two denominator rows, then
                DMA-broadcast of 1/d."""
                base = 2 * j * DSTRIDE
                da = normp.tile([64, 25], F32, tag="da", name=f"da{j}_{b}")
                nc.sync.dma_start(
                    da[:], dall_d[b, base:base + 1600]
                    .rearrange("(p f) -> p f", f=25))
                di = normp.tile([64, 25], BF16, tag="di", name=f"di{j}_{b}")
                with nc.allow_low_precision(reason="1/d broadcast in bf16"):
                    nc.vector.reciprocal(di[:], da[:])
                nc.sync.dma_start(
                    dinv_d[b, base:base + 1600]
                    .rearrange("(p f) -> p f", f=25), di[:])
                dr = normp.tile([128, W], BF16, tag="drep", name=f"dr{j}_{b}")
                for hh in range(2):
                    row = dinv_d[b, (2 * j + hh) * DSTRIDE:
                                 (2 * j + hh) * DSTRIDE + N]
                    src = bass.AP(tensor=row.tensor, offset=row.offset,
                                  ap=[[0, 64]] + row.ap)
                    nc.sync.dma_start(dr[hh * 64:(hh + 1) * 64, 0:N], src)
                nc.vector.tensor_tensor(
                    oT[j][:, 0:N], oT[j][:, 0:N], dr[:, 0:N],
                    mybir.AluOpType.mult)

            # ---------------- attention ----------------
            def emit_attention(b, qT, kT, vp, oT, fillers):
                """head-streamed: per (h, kc) step the PE stream carries
                S(h, kc) then O(h-1, kc); filler closures attached to (h, kc)
                run after that step's emission. fillers[(h, kc)] -> [fn]."""
                steps = [(h, kc) for h in range(H) for kc in range(NKC)]
                pts = {}
                psO = {}

                def issue_ebt(idx):
                    h, kc = steps[idx]
                    kr = _kr(kc)
                    t = expbp.tile([128, W], BF16, tag="expb",
                                   name=f"ebt{h}_{kc}_{b}")
                    eng = nc.gpsimd if (idx % 3 == 2) else nc.sync
                    eng.dma_start(t[0:kr, 0:N],
                                  expB_d[h, kc * 128:kc * 128 + kr, :])
                    return t

                ebt_q = {}
                for i in range(5):
                    ebt_q[i] = issue_ebt(i)

                def emit_O_step(h, kc):
                    kr = _kr(kc)
                    if kc == 0:
                        psO[h] = psum_o.tile([HD + 1, W], F32, tag="o",
                                             name=f"pso{h}_{b}")
                    pt = pts.pop((h, kc))
                    for (c0, cn) in CG_N:
                        nc.tensor.matmul(
                            psO[h][:, c0:c0 + cn],
                            vp[kc][0:kr, h * (HD + 1):(h + 1) * (HD + 1)],
                            pt[0:kr, c0:c0 + cn],
                            start=(kc == 0), stop=(kc == NKC - 1),
                        )

                def emit_O_evac(h):
                    j, hh = h // 2, h % 2
                    nc.vector.tensor_copy(
                        oT[j][hh * 64:hh * 64 + 64, 0:N], psO[h][0:64, 0:N])
                    dn = normp.tile([65, W], F32, tag="dn", name=f"dn{h}_{b}")
                    nc.vector.tensor_copy(dn[64:65, 0:W], psO[h][64:65, 0:W])
                    nc.sync.dma_start(
                        dall_d[b, h * DSTRIDE:h * DSTRIDE + W],
                        dn[64:65, 0:W])

                for i, (h, kc) in enumerate(steps):
                    j = h // 2
                    po = (h % 2) * 64
                    kr = _kr(kc)
                    # S matmuls
                    ps = psum_s.tile([128, W], F32, tag="s",
                                     name=f"pss{h}_{kc}_{b}")
                    for (c0, cn) in CG_N:
                        nc.tensor.matmul(
                            ps[0:kr, c0:c0 + cn],
                            kT[j][po:po + 64, kc * 128:kc * 128 + kr],
                            qT[j][po:po + 64, c0:c0 + cn],
                            start=True, stop=True,
                        )
                    # O for previous head rides in the same step
                    if h > 0:
                        emit_O_step(h - 1, kc)
                    # exp + expB multiply
                    es = flow.tile([128, W], BF16, tag="expS",
                                   name=f"es{h}_{kc}_{b}")
                    nc.scalar.activation(
                        es[0:kr, 0:W], ps[0:kr, 0:W],
                        mybir.ActivationFunctionType.Exp)
                    pt = ptp.tile([128, W], BF16, tag="pT",
                                  name=f"pt{h}_{kc}_{b}")
                    meng = nc.gpsimd if (i % 6 == 5) else nc.vector
                    meng.tensor_tensor(
                        pt[0:kr, 0:N], es[0:kr, 0:N], ebt_q.pop(i)[0:kr, 0:N],
                        mybir.AluOpType.mult)
                    pts[(h, kc)] = pt
                    if i + 5 < len(steps):
                        ebt_q[i + 5] = issue_ebt(i + 5)
                    # previous head's O evac at its boundary
                    if kc == NKC - 1 and h > 0:
                        emit_O_evac(h - 1)
                    for f in fillers.get((h, kc), []):
                        f()
                # trailing O convoy for the last head
                for kc in range(NKC):
                    emit_O_step(H - 1, kc)
                emit_O_evac(H - 1)
                for f in fillers.get((H, 0), []):
                    f()

            # ---------------- program ----------------
            xts0 = load_x(0)
            load_qkvw()
            qT0, kT0 = alloc_qkT(0)
            qT1, kT1 = alloc_qkT(1)
            # head phase: q0, k0, q1 + V0 kc0-2 (S-slot rotation, pre-attn)
            emit_qkT_convoy(0, 0, xts0, qT0, kT0, psum_s, "s")
            emit_qkT_convoy(0, NCC + 0, xts0, qT0, kT0, psum_s, "s")
            emit_qkT_convoy(0, 1, xts0, qT0, kT0, psum_s, "s")
            vp0 = alloc_vp(0)
            for kc in range(3):
                emit_v_convoy(0, kc, xts0, vp0, psum_s, "s")
            oT0 = [perb.tile([128, W], BF16, tag=f"oT{i}", name=f"oT{i}_0")
                   for i in range(NCC)]
            oT1 = [perb.tile([128, W], BF16, tag=f"oT{i}", name=f"oT{i}_1")
                   for i in range(NCC)]
            vp1 = alloc_vp(1)
            xts1_box = {}

            def qk0(oc):
                return lambda: emit_qkT_convoy(0, oc, xts0, qT0, kT0,
                                               psum_f, "f")

            def qk1(oc):
                return lambda: emit_qkT_convoy(1, oc, xts1_box[0], qT1, kT1,
                                               psum_f, "f")

            def v0(kc):
                return lambda: emit_v_convoy(0, kc, xts0, vp0, psum_f, "f")

            def v1(kc):
                return lambda: emit_v_convoy(1, kc, xts1_box[0], vp1,
                                             psum_f, "f")

            def load_x1():
                xts1_box[0] = load_x(1)

            KOF = NCC  # k output-chunk offset
            fill0 = {
                (0, 0): [v0(3)],
                (0, 1): [load_x1],
                (0, 2): [v0(4)],
                (0, 3): [qk0(KOF + 1)],          # k1 (needed h=2)
                (0, 4): [v0(5)],
                (0, 5): [load_pw],
                (0, 6): [v0(6)],
                (1, 2): [qk0(2)],                # q2 (h=4)
                (1, 5): [qk0(KOF + 2)],          # k2
                (2, 2): [qk0(3)],                # q3 (h=6)
                (2, 5): [qk0(KOF + 3)],          # k3
                (3, 2): [qk0(4)],                # q4 (h=8)
                (3, 5): [qk0(KOF + 4)],          # k4
                (4, 2): [qk0(5)],                # q5 (h=10)
                (4, 5): [qk0(KOF + 5)],          # k5
                (5, 2): [qk1(0)],
                (5, 5): [qk1(KOF + 0)],
                (6, 2): [v1(0)],
                (7, 2): [v1(1)],
                (8, 2): [v1(2)],
                (9, 2): [v1(3)],
                (10, 2): [v1(4)],
                (10, 5): [v1(5)],
                (11, 2): [qk1(1)],               # needed attn1 h=2
                (11, 5): [qk1(KOF + 1)],
            }

            def proj0(tt):
                return lambda: emit_proj_convoy(0, tt, oT0, psum_f, "f")

            def n0(j):
                return lambda: emit_norm_pair(0, j, oT0)

            def n1(j):
                return lambda: emit_norm_pair(1, j, oT1)

            fill1 = {
                (0, 1): [v1(6)],
                (0, 2): [n0(0)],
                (0, 3): [qk1(2)],                # needed h=4
                (0, 4): [n0(1)],
                (0, 6): [qk1(KOF + 2)],
                (1, 1): [n0(2)],
                (1, 2): [qk1(3)],                # h=6
                (1, 4): [n0(3)],
                (1, 5): [qk1(KOF + 3)],
                (2, 1): [n0(4)],
                (2, 2): [qk1(4)],                # h=8
                (2, 4): [n0(5)],
                (2, 5): [qk1(KOF + 4)],
                (3, 2): [qk1(5)],                # h=10
                (3, 5): [qk1(KOF + 5)],
                (4, 2): [proj0(0)],
                (5, 1): [n1(0)],
                (5, 2): [proj0(1)],
                (6, 2): [proj0(2)],
                (7, 1): [n1(1)],
                (7, 2): [proj0(3)],
                (8, 2): [proj0(4)],
                (9, 1): [n1(2)],
                (9, 2): [proj0(5)],
                (10, 2): [proj0(6)],
                (11, 1): [n1(3)],
                (H, 0): [n1(4)],
            }

            emit_attention(0, qT0, kT0, vp0, oT0, fill0)
            emit_attention(1, qT1, kT1, vp1, oT1, fill1)

            # tail: last normalize pair + batch-1 projection (alternate psum
            # tags so consecutive convoys don't serialize on one slot)
            emit_norm_pair(1, 5, oT1)
            for tt in range(NKC):
                if tt % 2 == 0:
                    emit_proj_convoy(1, tt, oT1, psum_s, "s")
                else:
                    emit_proj_convoy(1, tt, oT1, psum_f, "f")

    nc.compile()
    return nc


def _relative_position_index():
    coords = np.stack(np.meshgrid(np.arange(WX), np.arange(WY), indexing="ij"))
    cf = coords.reshape(2, -1)
    rel = cf[:, :, None] - cf[:, None, :]
    rel = rel.transpose(1, 2, 0).astype(np.int64)
    rel[:, :, 0] += WX - 1
    rel[:, :, 1] += WY - 1
    rel[:, :, 0] *= 2 * WY - 1
    return rel.sum(-1)  # [L, L]


def _host_prep(x, qkv_w, proj_w, proj_b, rel_table, g2l, g2g):
    x = np.asarray(x, np.float32)
    qkv_w = np.asarray(qkv_w, np.float32)
    proj_w = np.asarray(proj_w, np.float32)
    proj_b = np.asarray(proj_b, np.float32)
    rel_table = np.asarray(rel_table, np.float32)
    g2l = np.asarray(g2l, np.float32)
    g2g = np.asarray(g2g, np.float32)

    bf16 = ml_dtypes.bfloat16
    xT = np.ascontiguousarray(x.transpose(0, 2, 1)).astype(bf16)   # [B, C, N]
    qkv_wT = np.ascontiguousarray(qkv_w.T).copy()                  # [C, 3C]
    qkv_wT[:, :C] *= SCALE                                         # fold q scale
    qkv_wT = qkv_wT.astype(bf16)
    proj_wT = np.ascontiguousarray(proj_w.T).astype(bf16)          # [C, C]
    pb = proj_b.reshape(1, C).astype(bf16)

    # expB[h, k, q] = exp(bias[h, q, k]); exp applied at table granularity,
    # then expanded by the constant-index relative-position gather.
    ridx = _relative_position_index()
    et = np.exp(rel_table)                                         # [3025, H]
    eg2l = np.exp(g2l)                                             # [2, H, 1]
    eg2g = np.exp(g2g)                                             # [H, 1, 1]
    expB = np.empty((H, N, N), np.float32)
    expB[:, 1:, 1:] = et[ridx].transpose(2, 1, 0)                  # [H, k, q]
    expB[:, 0, 0] = eg2g[:, 0, 0]
    expB[:, 1:, 0] = eg2l[0][:, 0][None, :].T                      # global query
    expB[:, 0, 1:] = eg2l[1][:, 0][:, None]                        # global key
    expB16 = expB.astype(bf16)

    in_maps = []
    for i in range(N_CORES):
        in_maps.append({
            "xT": xT[i * B_LOC:(i + 1) * B_LOC],
            "qkv_wT": qkv_wT,
            "proj_wT": proj_wT,
            "proj_b": pb,
            "expB": expB16,
        })
    return in_maps


_NC = None


def get_nc():
    global _NC
    if _NC is None:
        _NC = build_nc()
    return _NC


def kernel(x, qkv_w, proj_w, proj_b, rel_table, g2l, g2g):
    in_maps = _host_prep(x, qkv_w, proj_w, proj_b, rel_table, g2l, g2g)
    nc = get_nc()
    res = run_bass_kernel_spmd(nc, in_maps, core_ids=list(range(N_CORES)))
    out = np.concatenate([res.results[i]["out"] for i in range(N_CORES)], axis=0)
    return out.astype(np.float32)


# revision 46
# speedup vs baseline: 1.1619x; 1.0286x over previous
"""Trainium2 Bass kernel for windowed/global sparse attention (Swin-style
relative-position bias + 1 global token), data-parallel over batch on 8 cores.

Shapes: B=16, N=785 (1 global + 28x28 local), C=768, H=12 heads, d=64.

Per-core device program (2 batches/core). Design notes:
  - qT/kT computed transposed ([d, tokens]) so S^T = K @ Q^T needs no
    transposes; v computed natural ([tokens, d]) with a ones column appended
    per head so the P @ V matmul also yields softmax denominators.
  - softmax: exp(S + bias) = exp(S) * expB with expB = exp(bias) gathered on
    host and shipped bf16; exp on ScalarE, multiply on VectorE (bf16 2x) with
    a fraction offloaded to GpSimd.
  - head-streamed schedule: for each (head h, key-chunk kc) step the TensorE
    stream carries S(h, kc) immediately followed by O(h-1, kc) — the dense
    O convoy rides inside the exp-paced S phase so the PE array never idles
    long enough for the HAM activity monitor to re-throttle the clock to
    1.2 GHz (43% of the old kernel's span ran cold).
  - PSUM discipline (8 banks): 2 rotating S slots + 1 O-convoy slot
    (allocated at first write) + 1 filler slot for qkv/v/proj convoys of the
    other/previous batch, which are spread between steps at (h, kc)
    granularity so no two convoys contend for the filler slot back-to-back.
  - denominators: O psum row 64 DMA'd straight to DRAM; reciprocal runs on
    a [128, 75]-reshaped view (0.5us instead of 5us at [12, 786]); 1/d is
    DMA-broadcast back (bf16) and multiplied into oT in place.
  - proj: bias applied via a ones-row matmul into the same psum accumulation,
    psum DMA'd straight to DRAM (no DVE add / evac).
"""

import numpy as np
import ml_dtypes

import concourse.bass as bass
import concourse.bacc as bacc
import concourse.tile as tile
from concourse import mybir
from concourse.bass_utils import run_bass_kernel_spmd

F32 = mybir.dt.float32
BF16 = mybir.dt.bfloat16
F8 = mybir.dt.float8e4
DR = mybir.MatmulPerfMode.DoubleRow

QS = 512.0   # host scale folded into q weight columns (with SCALE)
KS = 64.0    # host scale folded into k weight columns
VS = 64.0    # host scale folded into v weight columns

WX = WY = 28
NGLO = 1
H = 12
L = WX * WY            # 784
N = NGLO + L           # 785
C = 768
HD = C // H            # 64
SCALE = HD ** -0.5
B = 16
N_CORES = 8
B_LOC = B // N_CORES   # 2
NCC = C // 128         # 6 contraction chunks
NKC = (N + 127) // 128  # 7 key/token chunks (last = 17 rows)
W = 786                # padded free width for N-sized tiles (even)
DSTRIDE = 800          # flat stride for denominator rows in DRAM scratch
DPAD = 9600            # 12*800 = 128*75 for the reshaped reciprocal

CG_N = [(0, 512), (512, 274)]
CG_C = [(0, 512), (512, 256)]


def _kr(kc):
    return min(128, N - kc * 128)


def build_nc():
    nc = bacc.Bacc(None, target_bir_lowering=False)

    xT_d = nc.dram_tensor("xT", [B_LOC, C, N], BF16, kind="ExternalInput")
    x8_d = nc.dram_tensor("x8", [B_LOC, 3, 128, 2, N], F8, kind="ExternalInput")
    qkw8_d = nc.dram_tensor("qk_w8", [3, 2, 128, 2, C], F8,
                            kind="ExternalInput")
    vwT_d = nc.dram_tensor("v_wT", [C, C], BF16, kind="ExternalInput")
    pwT_d = nc.dram_tensor("proj_wT", [C, C], BF16, kind="ExternalInput")
    pb_d = nc.dram_tensor("proj_b", [1, C], BF16, kind="ExternalInput")
    expB_d = nc.dram_tensor("expB", [H, N, N], BF16, kind="ExternalInput")
    out_d = nc.dram_tensor("out", [B_LOC, N, C], F32, kind="ExternalOutput")
    dall_d = nc.dram_tensor("dall_scratch", [B_LOC, DPAD], F32)
    dinv_d = nc.dram_tensor("dinv_scratch", [B_LOC, DPAD], BF16)

    with tile.TileContext(nc) as tc:
        with (
            tc.tile_pool(name="consts", bufs=1) as consts,
            tc.tile_pool(name="perb", bufs=2) as perb,
            tc.tile_pool(name="expbp", bufs=8) as expbp,
            tc.tile_pool(name="flow", bufs=8) as flow,
            tc.tile_pool(name="ptp", bufs=16) as ptp,
            tc.tile_pool(name="norm", bufs=2) as normp,
            tc.tile_pool(name="outp", bufs=2) as outp,
            tc.tile_pool(name="psum_s", bufs=2, space=bass.MemorySpace.PSUM) as psum_s,
            tc.tile_pool(name="psum_o", bufs=1, space=bass.MemorySpace.PSUM) as psum_o,
            tc.tile_pool(name="psum_f", bufs=1, space=bass.MemorySpace.PSUM) as psum_f,
        ):
            # ---- resident weights ----
            # q/k weights as fp8 DoubleRow tiles, per (contraction-pair p,
            # block t in {q,k}): [128, 2, 768] — small pair-strides keep the
            # DoubleRow LDW AP legal. v weights stay bf16 (v-path noise
            # passes straight to the output; q/k noise is softmax-damped).
            qkw8 = [[consts.tile([128, 2, C], F8, tag=f"qkw{p}_{t}",
                                 name=f"qkw{p}_{t}") for t in range(2)]
                    for p in range(3)]
            vw16 = [consts.tile([128, C], BF16, tag=f"vw{cc}", name=f"vw{cc}")
                    for cc in range(NCC)]
            pw16 = [consts.tile([128, C], BF16, tag=f"pw{cc}", name=f"pw{cc}")
                    for cc in range(NCC)]
            pb16 = consts.tile([1, C], BF16, tag="pb16")
            ones = consts.tile([1, 128], BF16, tag="ones")
            nc.vector.memset(ones[:], 1.0)

            def load_qkvw():
                for p in range(3):
                    for t in range(2):
                        nc.gpsimd.dma_start(qkw8[p][t][:], qkw8_d[p, t])
                for cc in range(NCC):
                    nc.gpsimd.dma_start(
                        vw16[cc][:], vwT_d[cc * 128:(cc + 1) * 128, :])

            def load_pw():
                for cc in range(NCC):
                    nc.sync.dma_start(
                        pw16[cc][:], pwT_d[cc * 128:(cc + 1) * 128, :])
                nc.sync.dma_start(pb16[:], pb_d[:])

            XW = 800  # x8 tile pair-stride: 16-byte aligned for DoubleRow

            def load_x(b):
                """bf16 x tiles (V path + stationary) and fp8 pair tiles."""
                xts = []
                for cc in range(NCC):
                    t = perb.tile([128, W], BF16, tag=f"xt{cc}",
                                  name=f"xt{cc}_{b}")
                    nc.sync.dma_start(
                        t[:, 0:N], xT_d[b, cc * 128:(cc + 1) * 128, :])
                    xts.append(t)
                x8s = []
                for p in range(3):
                    t = perb.tile([128, 2, XW], F8, tag=f"x8t{p}",
                                  name=f"x8t{p}_{b}")
                    nc.sync.dma_start(t[:, :, 0:N], x8_d[b, p])
                    x8s.append(t)
                return xts, x8s

            def alloc_qkT(b):
                qT = [perb.tile([128, W], BF16, tag=f"qT{i}", name=f"qT{i}_{b}")
                      for i in range(NCC)]
                kT = [perb.tile([128, W], BF16, tag=f"kT{i}", name=f"kT{i}_{b}")
                      for i in range(NCC)]
                return qT, kT

            def emit_qkT_convoy(b, oc, x8s, qT, kT, pool, ptag):
                """one output chunk (128 cols of q or k), contraction over C
                via 3 fp8 DoubleRow matmuls per column group."""
                ps = pool.tile([128, W], F32, tag=ptag, name=f"psqk{oc}_{b}")
                blk, col = (0, oc * 128) if oc < NCC else (1, (oc - NCC) * 128)
                for p in range(3):
                    for (c0, cn) in CG_N:
                        nc.tensor.matmul(
                            ps[:, c0:c0 + cn],
                            qkw8[p][blk][:, :, col:col + 128],
                            x8s[p][:, :, c0:c0 + cn],
                            start=(p == 0), stop=(p == 2),
                            perf_mode=DR,
                        )
                dst = qT[oc] if oc < NCC else kT[oc - NCC]
                sc = 1.0 / QS if oc < NCC else 1.0 / KS
                nc.vector.tensor_scalar_mul(dst[:, 0:N], ps[:, 0:N], sc)

            def alloc_vp(b):
                return [perb.tile([128, H * (HD + 1)], BF16, tag=f"vp{i}",
                                  name=f"vp{i}_{b}") for i in range(NKC)]

            def emit_v_convoy(b, kc, xts, vp, pool, ptag):
                kr = _kr(kc)
                ps = pool.tile([128, W], F32, tag=ptag, name=f"psv{kc}_{b}")
                for cc in range(NCC):
                    for (c0, cn) in CG_C:
                        nc.tensor.matmul(
                            ps[0:kr, c0:c0 + cn],
                            xts[cc][:, kc * 128:kc * 128 + kr],
                            vw16[cc][:, c0:c0 + cn],
                            start=(cc == 0), stop=(cc == NCC - 1),
                        )
                v3 = vp[kc][:].rearrange("p (h e) -> p h e", e=HD + 1)
                nc.vector.tensor_copy(
                    v3[0:kr, :, 0:HD],
                    ps[0:kr, 0:C].rearrange("p (h d) -> p h d", d=HD),
                )
                nc.vector.memset(v3[0:kr, :, HD:HD + 1], 1.0)

            def emit_proj_part1(b, tt, oT, pool, ptag, ncc1):
                """bias + contraction chunks 0..ncc1-1, psum left open."""
                ts_ = _kr(tt)
                ps = pool.tile([128, W], F32, tag=ptag, name=f"psp{tt}_{b}")
                for (c0, cn) in CG_C:
                    nc.tensor.matmul(
                        ps[0:ts_, c0:c0 + cn],
                        ones[0:1, 0:ts_],
                        pb16[0:1, c0:c0 + cn],
                        start=True, stop=False,
                    )
                    for cc in range(ncc1):
                        nc.tensor.matmul(
                            ps[0:ts_, c0:c0 + cn],
                            oT[cc][:, tt * 128:tt * 128 + ts_],
                            pw16[cc][:, c0:c0 + cn],
                            start=False, stop=False,
                        )
                return ps

            def emit_proj_part2(b, tt, oT, ps, ncc1):
                ts_ = _kr(tt)
                for (c0, cn) in CG_C:
                    for cc in range(ncc1, NCC):
                        nc.tensor.matmul(
                            ps[0:ts_, c0:c0 + cn],
                            oT[cc][:, tt * 128:tt * 128 + ts_],
                            pw16[cc][:, c0:c0 + cn],
                            start=False, stop=(cc == NCC - 1),
                        )
                ob = outp.tile([128, C], F32, tag="ob", name=f"ob{tt}_{b}")
                nc.vector.tensor_copy(ob[0:ts_, :], ps[0:ts_, 0:C])
                nc.sync.dma_start(
                    out_d[b, tt * 128:tt * 128 + ts_, :], ob[0:ts_, :])

            def emit_proj_convoy(b, tt, oT, pool, ptag):
                """one token chunk of the projection, bias via ones-matmul."""
                ps = emit_proj_part1(b, tt, oT, pool, ptag, NCC - 1)
                emit_proj_part2(b, tt, oT, ps, NCC - 1)

            def emit_norm_pair(b, j, oT):
                """in-place oT[j] *= 1/d: per-pair reciprocal on a [64, 25]
                reshaped view of the pair's two denominator rows, then
                DMA-broadcast of 1/d."""
                base = 2 * j * DSTRIDE
                da = normp.tile([64, 25], F32, tag="da", name=f"da{j}_{b}")
                nc.sync.dma_start(
                    da[:], dall_d[b, base:base + 1600]
                    .rearrange("(p f) -> p f", f=25))
                di = normp.tile([64, 25], BF16, tag="di", name=f"di{j}_{b}")
                with nc.allow_low_precision(reason="1/d broadcast in bf16"):
                    nc.vector.reciprocal(di[:], da[:])
                nc.sync.dma_start(
                    dinv_d[b, base:base + 1600]
                    .rearrange("(p f) -> p f", f=25), di[:])
                dr = normp.tile([128, W], BF16, tag="drep", name=f"dr{j}_{b}")
                for hh in range(2):
                    row = dinv_d[b, (2 * j + hh) * DSTRIDE:
                                 (2 * j + hh) * DSTRIDE + N]
                    src = bass.AP(tensor=row.tensor, offset=row.offset,
                                  ap=[[0, 64]] + row.ap)
                    nc.sync.dma_start(dr[hh * 64:(hh + 1) * 64, 0:N], src)
                nc.vector.tensor_tensor(
                    oT[j][:, 0:N], oT[j][:, 0:N], dr[:, 0:N],
                    mybir.AluOpType.mult)

            # ---------------- attention ----------------
            def emit_attention(b, qT, kT, vp, oT, fillers):
                """head-streamed: per (h, kc) step the PE stream carries
                S(h, kc) then O(h-1, kc); filler closures attached to (h, kc)
                run after that step's emission. fillers[(h, kc)] -> [fn]."""
                steps = [(h, kc) for h in range(H) for kc in range(NKC)]
                pts = {}
                psO = {}

                def issue_ebt(idx):
                    h, kc = steps[idx]
                    kr = _kr(kc)
                    t = expbp.tile([128, W], BF16, tag="expb",
                                   name=f"ebt{h}_{kc}_{b}")
                    eng = nc.gpsimd if (idx % 3 == 2) else nc.sync
                    eng.dma_start(t[0:kr, 0:N],
                                  expB_d[h, kc * 128:kc * 128 + kr, :])
                    return t

                ebt_q = {}
                for i in range(5):
                    ebt_q[i] = issue_ebt(i)

                def emit_O_step(h, kc):
                    kr = _kr(kc)
                    if kc == 0:
                        psO[h] = psum_o.tile([HD + 1, W], F32, tag="o",
                                             name=f"pso{h}_{b}")
                    pt = pts.pop((h, kc))
                    for (c0, cn) in CG_N:
                        nc.tensor.matmul(
                            psO[h][:, c0:c0 + cn],
                            vp[kc][0:kr, h * (HD + 1):(h + 1) * (HD + 1)],
                            pt[0:kr, c0:c0 + cn],
                            start=(kc == 0), stop=(kc == NKC - 1),
                        )

                def emit_O_evac(h):
                    j, hh = h // 2, h % 2
                    nc.vector.tensor_copy(
                        oT[j][hh * 64:hh * 64 + 64, 0:N], psO[h][0:64, 0:N])
                    dn = normp.tile([65, W], F32, tag="dn", name=f"dn{h}_{b}")
                    nc.vector.tensor_copy(dn[64:65, 0:W], psO[h][64:65, 0:W])
                    nc.sync.dma_start(
                        dall_d[b, h * DSTRIDE:h * DSTRIDE + W],
                        dn[64:65, 0:W])

                for i, (h, kc) in enumerate(steps):
                    j = h // 2
                    po = (h % 2) * 64
                    kr = _kr(kc)
                    # S matmuls
                    ps = psum_s.tile([128, W], F32, tag="s",
                                     name=f"pss{h}_{kc}_{b}")
                    for (c0, cn) in CG_N:
                        nc.tensor.matmul(
                            ps[0:kr, c0:c0 + cn],
                            kT[j][po:po + 64, kc * 128:kc * 128 + kr],
                            qT[j][po:po + 64, c0:c0 + cn],
                            start=True, stop=True,
                        )
                    # O for previous head rides in the same step
                    if h > 0:
                        emit_O_step(h - 1, kc)
                    # exp + expB multiply
                    es = flow.tile([128, W], BF16, tag="expS",
                                   name=f"es{h}_{kc}_{b}")
                    nc.scalar.activation(
                        es[0:kr, 0:W], ps[0:kr, 0:W],
                        mybir.ActivationFunctionType.Exp)
                    pt = ptp.tile([128, W], BF16, tag="pT",
                                  name=f"pt{h}_{kc}_{b}")
                    meng = nc.gpsimd if (i % 3 == 2) else nc.vector
                    meng.tensor_tensor(
                        pt[0:kr, 0:N], es[0:kr, 0:N], ebt_q.pop(i)[0:kr, 0:N],
                        mybir.AluOpType.mult)
                    pts[(h, kc)] = pt
                    if i + 5 < len(steps):
                        ebt_q[i + 5] = issue_ebt(i + 5)
                    # previous head's O evac at its boundary
                    if kc == NKC - 1 and h > 0:
                        emit_O_evac(h - 1)
                    for f in fillers.get((h, kc), []):
                        f()
                # trailing O convoy for the last head
                for kc in range(NKC):
                    emit_O_step(H - 1, kc)
                emit_O_evac(H - 1)
                for f in fillers.get((H, 0), []):
                    f()

            # ---------------- program ----------------
            xts0, x8s0 = load_x(0)
            load_qkvw()
            qT0, kT0 = alloc_qkT(0)
            qT1, kT1 = alloc_qkT(1)
            # head phase: q0, k0, q1 + V0 kc0-2 (S-slot rotation, pre-attn)
            emit_qkT_convoy(0, 0, x8s0, qT0, kT0, psum_s, "s")
            emit_qkT_convoy(0, NCC + 0, x8s0, qT0, kT0, psum_s, "s")
            emit_qkT_convoy(0, 1, x8s0, qT0, kT0, psum_s, "s")
            vp0 = alloc_vp(0)
            for kc in range(3):
                emit_v_convoy(0, kc, xts0, vp0, psum_s, "s")
            oT0 = [perb.tile([128, W], BF16, tag=f"oT{i}", name=f"oT{i}_0")
                   for i in range(NCC)]
            oT1 = [perb.tile([128, W], BF16, tag=f"oT{i}", name=f"oT{i}_1")
                   for i in range(NCC)]
            vp1 = alloc_vp(1)
            xts1_box = {}

            def qk0(oc):
                return lambda: emit_qkT_convoy(0, oc, x8s0, qT0, kT0,
                                               psum_f, "f")

            def qk1(oc):
                return lambda: emit_qkT_convoy(1, oc, xts1_box[1], qT1, kT1,
                                               psum_f, "f")

            def v0(kc):
                return lambda: emit_v_convoy(0, kc, xts0, vp0, psum_f, "f")

            def v1(kc):
                return lambda: emit_v_convoy(1, kc, xts1_box[0], vp1,
                                             psum_f, "f")

            def load_x1():
                xts1_box[0], xts1_box[1] = load_x(1)

            KOF = NCC  # k output-chunk offset
            fill0 = {
                (0, 0): [v0(3)],
                (0, 1): [load_x1],
                (0, 2): [v0(4)],
                (0, 3): [qk0(KOF + 1)],          # k1 (needed h=2)
                (0, 4): [v0(5)],
                (0, 5): [load_pw],
                (0, 6): [v0(6)],
                (1, 2): [qk0(2)],                # q2 (h=4)
                (1, 5): [qk0(KOF + 2)],          # k2
                (2, 2): [qk0(3)],                # q3 (h=6)
                (2, 5): [qk0(KOF + 3)],          # k3
                (3, 2): [qk0(4)],                # q4 (h=8)
                (3, 5): [qk0(KOF + 4)],          # k4
                (4, 2): [qk0(5)],                # q5 (h=10)
                (4, 5): [qk0(KOF + 5)],          # k5
                (5, 2): [qk1(0)],
                (5, 5): [qk1(KOF + 0)],
                (6, 2): [v1(0)],
                (7, 2): [v1(1)],
                (8, 2): [v1(2)],
                (9, 2): [v1(3)],
                (10, 2): [v1(4)],
                (10, 5): [v1(5)],
                (11, 2): [qk1(1)],               # needed attn1 h=2
                (11, 5): [qk1(KOF + 1)],
            }

            def proj0(tt):
                return lambda: emit_proj_convoy(0, tt, oT0, psum_f, "f")

            def n0(j):
                return lambda: emit_norm_pair(0, j, oT0)

            def n1(j):
                return lambda: emit_norm_pair(1, j, oT1)

            fill1 = {
                (0, 1): [v1(6)],
                (0, 2): [n0(0)],
                (0, 3): [qk1(2)],                # needed h=4
                (0, 4): [n0(1)],
                (0, 6): [qk1(KOF + 2)],
                (1, 1): [n0(2)],
                (1, 2): [qk1(3)],                # h=6
                (1, 4): [n0(3)],
                (1, 5): [qk1(KOF + 3)],
                (2, 1): [n0(4)],
                (2, 2): [qk1(4)],                # h=8
                (2, 4): [n0(5)],
                (2, 5): [qk1(KOF + 4)],
                (3, 2): [qk1(5)],                # h=10
                (3, 5): [qk1(KOF + 5)],
                (4, 2): [proj0(0)],
                (5, 1): [n1(0)],
                (5, 2): [proj0(1)],
                (6, 2): [proj0(2)],
                (7, 1): [n1(1)],
                (7, 2): [proj0(3)],
                (8, 2): [proj0(4)],
                (9, 1): [n1(2)],
                (9, 2): [proj0(5)],
                (10, 2): [proj0(6)],
                (11, 1): [n1(3)],
                (H, 0): [n1(4)],
            }

            emit_attention(0, qT0, kT0, vp0, oT0, fill0)
            emit_attention(1, qT1, kT1, vp1, oT1, fill1)

            # tail: last normalize pair's DMA chain hides under split proj
            # accumulation — contraction chunks 0-4 (pairs already normalized)
            # run across 4 open psum slots while pair 5's 1/d lands; chunk 5
            # joins in part2.
            emit_norm_pair(1, 5, oT1)
            tail_ps = {}
            tail_pool = [(psum_s, "s"), (psum_f, "f"),
                         (psum_o, "o"), (psum_s, "s")]
            for tt in range(4):
                pool, ptag = tail_pool[tt]
                tail_ps[tt] = emit_proj_part1(1, tt, oT1, pool, ptag, NCC - 1)
            for tt in range(4):
                emit_proj_part2(1, tt, oT1, tail_ps[tt], NCC - 1)
            for i, tt in enumerate(range(4, NKC)):
                pool, ptag = [(psum_f, "f"), (psum_o, "o"),
                              (psum_s, "s")][i % 3]
                emit_proj_convoy(1, tt, oT1, pool, ptag)

    nc.compile()
    return nc


def _relative_position_index():
    coords = np.stack(np.meshgrid(np.arange(WX), np.arange(WY), indexing="ij"))
    cf = coords.reshape(2, -1)
    rel = cf[:, :, None] - cf[:, None, :]
    rel = rel.transpose(1, 2, 0).astype(np.int64)
    rel[:, :, 0] += WX - 1
    rel[:, :, 1] += WY - 1
    rel[:, :, 0] *= 2 * WY - 1
    return rel.sum(-1)  # [L, L]


def _host_prep(x, qkv_w, proj_w, proj_b, rel_table, g2l, g2g):
    x = np.asarray(x, np.float32)
    qkv_w = np.asarray(qkv_w, np.float32)
    proj_w = np.asarray(proj_w, np.float32)
    proj_b = np.asarray(proj_b, np.float32)
    rel_table = np.asarray(rel_table, np.float32)
    g2l = np.asarray(g2l, np.float32)
    g2g = np.asarray(g2g, np.float32)

    bf16 = ml_dtypes.bfloat16
    f8 = ml_dtypes.float8_e4m3fn
    # x and q/k weights ship as fp8e4 in the [.., 128, 2, *] DoubleRow
    # layout: contraction chunk p covers C-rows [256p, 256p+256), subtile
    # s = rows [256p+128s, +128). Weight columns pre-scaled into fp8's
    # normal range; the psum evacuation rescales by 1/QS, 1/KS.
    xT = x.transpose(0, 2, 1)                                      # [B, C, N]
    xT16 = np.ascontiguousarray(xT).astype(bf16)
    x8 = np.clip(xT, -240, 240).astype(f8)
    x8 = np.ascontiguousarray(
        x8.reshape(B, 3, 2, 128, N).transpose(0, 1, 3, 2, 4))     # [B,3,128,2,N]
    qk_wT = qkv_w[:2 * C].T.copy()                                 # [C, 2C]
    qk_wT[:, :C] *= SCALE * QS
    qk_wT[:, C:] *= KS
    w8 = np.clip(qk_wT, -240, 240).astype(f8)
    # [C, 2C] -> [ccp 3, block 2, 128, 2, 768]
    w8 = np.ascontiguousarray(
        w8.reshape(3, 2, 128, 2, C).transpose(0, 3, 2, 1, 4))
    v_wT = np.ascontiguousarray(qkv_w[2 * C:].T).astype(bf16)      # [C, C]
    proj_wT = np.ascontiguousarray(proj_w.T).astype(bf16)          # [C, C]
    pb = proj_b.reshape(1, C).astype(bf16)

    # expB[h, k, q] = exp(bias[h, q, k]); exp applied at table granularity,
    # then expanded by the constant-index relative-position gather.
    ridx = _relative_position_index()
    et = np.exp(rel_table)                                         # [3025, H]
    eg2l = np.exp(g2l)                                             # [2, H, 1]
    eg2g = np.exp(g2g)                                             # [H, 1, 1]
    expB = np.empty((H, N, N), np.float32)
    expB[:, 1:, 1:] = et[ridx].transpose(2, 1, 0)                  # [H, k, q]
    expB[:, 0, 0] = eg2g[:, 0, 0]
    expB[:, 1:, 0] = eg2l[0][:, 0][None, :].T                      # global query
    expB[:, 0, 1:] = eg2l[1][:, 0][:, None]                        # global key
    expB16 = expB.astype(bf16)

    in_maps = []
    for i in range(N_CORES):
        in_maps.append({
            "xT": xT16[i * B_LOC:(i + 1) * B_LOC],
            "x8": x8[i * B_LOC:(i + 1) * B_LOC],
            "qk_w8": w8,
            "v_wT": v_wT,
            "proj_wT": proj_wT,
            "proj_b": pb,
            "expB": expB16,
        })
    return in_maps


_NC = None


def get_nc():
    global _NC
    if _NC is None:
        _NC = build_nc()
    return _NC


def kernel(x, qkv_w, proj_w, proj_b, rel_table, g2l, g2g):
    in_maps = _host_prep(x, qkv_w, proj_w, proj_b, rel_table, g2l, g2g)
    nc = get_nc()
    res = run_bass_kernel_spmd(nc, in_maps, core_ids=list(range(N_CORES)))
    out = np.concatenate([res.results[i]["out"] for i in range(N_CORES)], axis=0)
    return out.astype(np.float32)


# revision 47
# speedup vs baseline: 1.2068x; 1.0386x over previous
"""Trainium2 Bass kernel for windowed/global sparse attention (Swin-style
relative-position bias + 1 global token), data-parallel over batch on 8 cores.

Shapes: B=16, N=785 (1 global + 28x28 local), C=768, H=12 heads, d=64.

Per-core device program (2 batches/core). Design notes:
  - qT/kT computed transposed ([d, tokens]) so S^T = K @ Q^T needs no
    transposes; v computed natural ([tokens, d]) with a ones column appended
    per head so the P @ V matmul also yields softmax denominators.
  - softmax: exp(S + bias) = exp(S) * expB with expB = exp(bias) gathered on
    host and shipped bf16; exp on ScalarE, multiply on VectorE (bf16 2x) with
    a fraction offloaded to GpSimd.
  - head-streamed schedule: for each (head h, key-chunk kc) step the TensorE
    stream carries S(h, kc) immediately followed by O(h-1, kc) — the dense
    O convoy rides inside the exp-paced S phase so the PE array never idles
    long enough for the HAM activity monitor to re-throttle the clock to
    1.2 GHz (43% of the old kernel's span ran cold).
  - PSUM discipline (8 banks): 2 rotating S slots + 1 O-convoy slot
    (allocated at first write) + 1 filler slot for qkv/v/proj convoys of the
    other/previous batch, which are spread between steps at (h, kc)
    granularity so no two convoys contend for the filler slot back-to-back.
  - denominators: O psum row 64 DMA'd straight to DRAM; reciprocal runs on
    a [128, 75]-reshaped view (0.5us instead of 5us at [12, 786]); 1/d is
    DMA-broadcast back (bf16) and multiplied into oT in place.
  - proj: bias applied via a ones-row matmul into the same psum accumulation,
    psum DMA'd straight to DRAM (no DVE add / evac).
"""

import numpy as np
import ml_dtypes

import concourse.bass as bass
import concourse.bacc as bacc
import concourse.tile as tile
from concourse import mybir
from concourse.bass_utils import run_bass_kernel_spmd

F32 = mybir.dt.float32
BF16 = mybir.dt.bfloat16
F8 = mybir.dt.float8e4
DR = mybir.MatmulPerfMode.DoubleRow

QS = 512.0   # host scale folded into q weight columns (with SCALE)
KS = 64.0    # host scale folded into k weight columns
VS = 64.0    # host scale folded into v weight columns

WX = WY = 28
NGLO = 1
H = 12
L = WX * WY            # 784
N = NGLO + L           # 785
C = 768
HD = C // H            # 64
SCALE = HD ** -0.5
B = 16
N_CORES = 8
B_LOC = B // N_CORES   # 2
NCC = C // 128         # 6 contraction chunks
NKC = (N + 127) // 128  # 7 key/token chunks (last = 17 rows)
W = 786                # padded free width for N-sized tiles (even)
DSTRIDE = 800          # flat stride for denominator rows in DRAM scratch
DPAD = 9600            # 12*800 = 128*75 for the reshaped reciprocal

CG_N = [(0, 512), (512, 274)]
CG_C = [(0, 512), (512, 256)]


def _kr(kc):
    return min(128, N - kc * 128)


def build_nc():
    nc = bacc.Bacc(None, target_bir_lowering=False)

    xT_d = nc.dram_tensor("xT", [B_LOC, C, N], BF16, kind="ExternalInput")
    x8_d = nc.dram_tensor("x8", [B_LOC, 3, 128, 2, N], F8, kind="ExternalInput")
    qkw8_d = nc.dram_tensor("qk_w8", [3, 2, 128, 2, C], F8,
                            kind="ExternalInput")
    vwT_d = nc.dram_tensor("v_wT", [C, C], BF16, kind="ExternalInput")
    pwT_d = nc.dram_tensor("proj_wT", [C, C], BF16, kind="ExternalInput")
    pb_d = nc.dram_tensor("proj_b", [1, C], BF16, kind="ExternalInput")
    expB_d = nc.dram_tensor("expB", [H, N, N], BF16, kind="ExternalInput")
    out_d = nc.dram_tensor("out", [B_LOC, N, C], F32, kind="ExternalOutput")
    dall_d = nc.dram_tensor("dall_scratch", [B_LOC, DPAD], F32)
    dinv_d = nc.dram_tensor("dinv_scratch", [B_LOC, DPAD], BF16)

    with tile.TileContext(nc) as tc:
        with (
            tc.tile_pool(name="consts", bufs=1) as consts,
            tc.tile_pool(name="perb", bufs=2) as perb,
            tc.tile_pool(name="expbp", bufs=8) as expbp,
            tc.tile_pool(name="flow", bufs=8) as flow,
            tc.tile_pool(name="ptp", bufs=16) as ptp,
            tc.tile_pool(name="norm", bufs=2) as normp,
            tc.tile_pool(name="outp", bufs=2) as outp,
            tc.tile_pool(name="psum_s", bufs=2, space=bass.MemorySpace.PSUM) as psum_s,
            tc.tile_pool(name="psum_o", bufs=1, space=bass.MemorySpace.PSUM) as psum_o,
            tc.tile_pool(name="psum_f", bufs=1, space=bass.MemorySpace.PSUM) as psum_f,
        ):
            # ---- resident weights ----
            # q/k weights as fp8 DoubleRow tiles, per (contraction-pair p,
            # block t in {q,k}): [128, 2, 768] — small pair-strides keep the
            # DoubleRow LDW AP legal. v weights stay bf16 (v-path noise
            # passes straight to the output; q/k noise is softmax-damped).
            qkw8 = [[consts.tile([128, 2, C], F8, tag=f"qkw{p}_{t}",
                                 name=f"qkw{p}_{t}") for t in range(2)]
                    for p in range(3)]
            vw16 = [consts.tile([128, C], BF16, tag=f"vw{cc}", name=f"vw{cc}")
                    for cc in range(NCC)]
            pw16 = [consts.tile([128, C], BF16, tag=f"pw{cc}", name=f"pw{cc}")
                    for cc in range(NCC)]
            pb16 = consts.tile([1, C], BF16, tag="pb16")
            ones = consts.tile([1, 128], BF16, tag="ones")
            nc.vector.memset(ones[:], 1.0)

            def load_qkvw():
                for p in range(3):
                    for t in range(2):
                        nc.gpsimd.dma_start(qkw8[p][t][:], qkw8_d[p, t])
                for cc in range(NCC):
                    nc.gpsimd.dma_start(
                        vw16[cc][:], vwT_d[cc * 128:(cc + 1) * 128, :])

            def load_pw():
                for cc in range(NCC):
                    nc.sync.dma_start(
                        pw16[cc][:], pwT_d[cc * 128:(cc + 1) * 128, :])
                nc.sync.dma_start(pb16[:], pb_d[:])

            XW = 800  # x8 tile pair-stride: 16-byte aligned for DoubleRow

            def load_x(b):
                """fp8 pair tiles first (q/k convoys consume them first);
                bf16 x via the scalar queue for batch 0 (idle pre-attention)
                so the sync queue reaches the ebt prefetches quickly."""
                x8s = []
                for p in range(3):
                    t = perb.tile([128, 2, XW], F8, tag=f"x8t{p}",
                                  name=f"x8t{p}_{b}")
                    nc.sync.dma_start(t[:, :, 0:N], x8_d[b, p])
                    x8s.append(t)
                xeng = nc.scalar if b == 0 else nc.sync
                xts = []
                for cc in range(NCC):
                    t = perb.tile([128, W], BF16, tag=f"xt{cc}",
                                  name=f"xt{cc}_{b}")
                    xeng.dma_start(
                        t[:, 0:N], xT_d[b, cc * 128:(cc + 1) * 128, :])
                    xts.append(t)
                return xts, x8s

            def alloc_qkT(b):
                qT = [perb.tile([128, W], BF16, tag=f"qT{i}", name=f"qT{i}_{b}")
                      for i in range(NCC)]
                kT = [perb.tile([128, W], BF16, tag=f"kT{i}", name=f"kT{i}_{b}")
                      for i in range(NCC)]
                return qT, kT

            def emit_qkT_convoy(b, oc, x8s, qT, kT, pool, ptag):
                """one output chunk (128 cols of q or k), contraction over C
                via 3 fp8 DoubleRow matmuls per column group."""
                ps = pool.tile([128, W], F32, tag=ptag, name=f"psqk{oc}_{b}")
                blk, col = (0, oc * 128) if oc < NCC else (1, (oc - NCC) * 128)
                for p in range(3):
                    for (c0, cn) in CG_N:
                        nc.tensor.matmul(
                            ps[:, c0:c0 + cn],
                            qkw8[p][blk][:, :, col:col + 128],
                            x8s[p][:, :, c0:c0 + cn],
                            start=(p == 0), stop=(p == 2),
                            perf_mode=DR,
                        )
                dst = qT[oc] if oc < NCC else kT[oc - NCC]
                sc = 1.0 / QS if oc < NCC else 1.0 / KS
                nc.vector.tensor_scalar_mul(dst[:, 0:N], ps[:, 0:N], sc)

            def alloc_vp(b):
                return [perb.tile([128, H * (HD + 1)], BF16, tag=f"vp{i}",
                                  name=f"vp{i}_{b}") for i in range(NKC)]

            def emit_v_convoy(b, kc, xts, vp, pool, ptag):
                kr = _kr(kc)
                ps = pool.tile([128, W], F32, tag=ptag, name=f"psv{kc}_{b}")
                for cc in range(NCC):
                    for (c0, cn) in CG_C:
                        nc.tensor.matmul(
                            ps[0:kr, c0:c0 + cn],
                            xts[cc][:, kc * 128:kc * 128 + kr],
                            vw16[cc][:, c0:c0 + cn],
                            start=(cc == 0), stop=(cc == NCC - 1),
                        )
                v3 = vp[kc][:].rearrange("p (h e) -> p h e", e=HD + 1)
                nc.vector.tensor_copy(
                    v3[0:kr, :, 0:HD],
                    ps[0:kr, 0:C].rearrange("p (h d) -> p h d", d=HD),
                )
                nc.vector.memset(v3[0:kr, :, HD:HD + 1], 1.0)

            def emit_proj_part1(b, tt, oT, pool, ptag, ncc1):
                """bias + contraction chunks 0..ncc1-1, psum left open."""
                ts_ = _kr(tt)
                ps = pool.tile([128, W], F32, tag=ptag, name=f"psp{tt}_{b}")
                for (c0, cn) in CG_C:
                    nc.tensor.matmul(
                        ps[0:ts_, c0:c0 + cn],
                        ones[0:1, 0:ts_],
                        pb16[0:1, c0:c0 + cn],
                        start=True, stop=False,
                    )
                    for cc in range(ncc1):
                        nc.tensor.matmul(
                            ps[0:ts_, c0:c0 + cn],
                            oT[cc][:, tt * 128:tt * 128 + ts_],
                            pw16[cc][:, c0:c0 + cn],
                            start=False, stop=False,
                        )
                return ps

            def emit_proj_part2(b, tt, oT, ps, ncc1):
                ts_ = _kr(tt)
                for (c0, cn) in CG_C:
                    for cc in range(ncc1, NCC):
                        nc.tensor.matmul(
                            ps[0:ts_, c0:c0 + cn],
                            oT[cc][:, tt * 128:tt * 128 + ts_],
                            pw16[cc][:, c0:c0 + cn],
                            start=False, stop=(cc == NCC - 1),
                        )
                ob = outp.tile([128, C], F32, tag="ob", name=f"ob{tt}_{b}")
                nc.vector.tensor_copy(ob[0:ts_, :], ps[0:ts_, 0:C])
                nc.sync.dma_start(
                    out_d[b, tt * 128:tt * 128 + ts_, :], ob[0:ts_, :])

            def emit_proj_convoy(b, tt, oT, pool, ptag):
                """one token chunk of the projection, bias via ones-matmul."""
                ps = emit_proj_part1(b, tt, oT, pool, ptag, NCC - 1)
                emit_proj_part2(b, tt, oT, ps, NCC - 1)

            def emit_norm_pair(b, j, oT):
                """in-place oT[j] *= 1/d: per-pair reciprocal on a [64, 25]
                reshaped view of the pair's two denominator rows, then
                DMA-broadcast of 1/d."""
                base = 2 * j * DSTRIDE
                da = normp.tile([64, 25], F32, tag="da", name=f"da{j}_{b}")
                nc.sync.dma_start(
                    da[:], dall_d[b, base:base + 1600]
                    .rearrange("(p f) -> p f", f=25))
                di = normp.tile([64, 25], BF16, tag="di", name=f"di{j}_{b}")
                with nc.allow_low_precision(reason="1/d broadcast in bf16"):
                    nc.vector.reciprocal(di[:], da[:])
                nc.sync.dma_start(
                    dinv_d[b, base:base + 1600]
                    .rearrange("(p f) -> p f", f=25), di[:])
                dr = normp.tile([128, W], BF16, tag="drep", name=f"dr{j}_{b}")
                for hh in range(2):
                    row = dinv_d[b, (2 * j + hh) * DSTRIDE:
                                 (2 * j + hh) * DSTRIDE + N]
                    src = bass.AP(tensor=row.tensor, offset=row.offset,
                                  ap=[[0, 64]] + row.ap)
                    nc.sync.dma_start(dr[hh * 64:(hh + 1) * 64, 0:N], src)
                nc.vector.tensor_tensor(
                    oT[j][:, 0:N], oT[j][:, 0:N], dr[:, 0:N],
                    mybir.AluOpType.mult)

            # ---------------- attention ----------------
            def emit_attention(b, qT, kT, vp, oT, fillers):
                """head-streamed: per (h, kc) step the PE stream carries
                S(h, kc) then O(h-1, kc); filler closures attached to (h, kc)
                run after that step's emission. fillers[(h, kc)] -> [fn]."""
                steps = [(h, kc) for h in range(H) for kc in range(NKC)]
                pts = {}
                psO = {}

                def issue_ebt(idx):
                    h, kc = steps[idx]
                    kr = _kr(kc)
                    t = expbp.tile([128, W], BF16, tag="expb",
                                   name=f"ebt{h}_{kc}_{b}")
                    eng = nc.gpsimd if (idx % 3 == 2) else nc.sync
                    eng.dma_start(t[0:kr, 0:N],
                                  expB_d[h, kc * 128:kc * 128 + kr, :])
                    return t

                ebt_q = {}
                for i in range(5):
                    ebt_q[i] = issue_ebt(i)

                def emit_O_step(h, kc):
                    kr = _kr(kc)
                    if kc == 0:
                        psO[h] = psum_o.tile([HD + 1, W], F32, tag="o",
                                             name=f"pso{h}_{b}")
                    pt = pts.pop((h, kc))
                    for (c0, cn) in CG_N:
                        nc.tensor.matmul(
                            psO[h][:, c0:c0 + cn],
                            vp[kc][0:kr, h * (HD + 1):(h + 1) * (HD + 1)],
                            pt[0:kr, c0:c0 + cn],
                            start=(kc == 0), stop=(kc == NKC - 1),
                        )

                def emit_O_evac(h):
                    j, hh = h // 2, h % 2
                    nc.vector.tensor_copy(
                        oT[j][hh * 64:hh * 64 + 64, 0:N], psO[h][0:64, 0:N])
                    dn = normp.tile([65, W], F32, tag="dn", name=f"dn{h}_{b}")
                    nc.vector.tensor_copy(dn[64:65, 0:W], psO[h][64:65, 0:W])
                    nc.sync.dma_start(
                        dall_d[b, h * DSTRIDE:h * DSTRIDE + W],
                        dn[64:65, 0:W])

                for i, (h, kc) in enumerate(steps):
                    j = h // 2
                    po = (h % 2) * 64
                    kr = _kr(kc)
                    # S matmuls
                    ps = psum_s.tile([128, W], F32, tag="s",
                                     name=f"pss{h}_{kc}_{b}")
                    for (c0, cn) in CG_N:
                        nc.tensor.matmul(
                            ps[0:kr, c0:c0 + cn],
                            kT[j][po:po + 64, kc * 128:kc * 128 + kr],
                            qT[j][po:po + 64, c0:c0 + cn],
                            start=True, stop=True,
                        )
                    # O for previous head rides in the same step
                    if h > 0:
                        emit_O_step(h - 1, kc)
                    # exp + expB multiply
                    es = flow.tile([128, W], BF16, tag="expS",
                                   name=f"es{h}_{kc}_{b}")
                    nc.scalar.activation(
                        es[0:kr, 0:W], ps[0:kr, 0:W],
                        mybir.ActivationFunctionType.Exp)
                    pt = ptp.tile([128, W], BF16, tag="pT",
                                  name=f"pt{h}_{kc}_{b}")
                    meng = nc.gpsimd if (i % 3 == 2) else nc.vector
                    meng.tensor_tensor(
                        pt[0:kr, 0:N], es[0:kr, 0:N], ebt_q.pop(i)[0:kr, 0:N],
                        mybir.AluOpType.mult)
                    pts[(h, kc)] = pt
                    if i + 5 < len(steps):
                        ebt_q[i + 5] = issue_ebt(i + 5)
                    # previous head's O evac at its boundary
                    if kc == NKC - 1 and h > 0:
                        emit_O_evac(h - 1)
                    for f in fillers.get((h, kc), []):
                        f()
                # trailing O convoy for the last head
                for kc in range(NKC):
                    emit_O_step(H - 1, kc)
                emit_O_evac(H - 1)
                for f in fillers.get((H, 0), []):
                    f()

            # ---------------- program ----------------
            xts0, x8s0 = load_x(0)
            load_qkvw()
            qT0, kT0 = alloc_qkT(0)
            qT1, kT1 = alloc_qkT(1)
            # head phase: q0, k0, q1 + V0 kc0-2 (S-slot rotation, pre-attn)
            emit_qkT_convoy(0, 0, x8s0, qT0, kT0, psum_s, "s")
            emit_qkT_convoy(0, NCC + 0, x8s0, qT0, kT0, psum_s, "s")
            emit_qkT_convoy(0, 1, x8s0, qT0, kT0, psum_s, "s")
            vp0 = alloc_vp(0)
            for kc in range(3):
                emit_v_convoy(0, kc, xts0, vp0, psum_s, "s")
            oT0 = [perb.tile([128, W], BF16, tag=f"oT{i}", name=f"oT{i}_0")
                   for i in range(NCC)]
            oT1 = [perb.tile([128, W], BF16, tag=f"oT{i}", name=f"oT{i}_1")
                   for i in range(NCC)]
            vp1 = alloc_vp(1)
            xts1_box = {}

            def qk0(oc):
                return lambda: emit_qkT_convoy(0, oc, x8s0, qT0, kT0,
                                               psum_f, "f")

            def qk1(oc):
                return lambda: emit_qkT_convoy(1, oc, xts1_box[1], qT1, kT1,
                                               psum_f, "f")

            def v0(kc):
                return lambda: emit_v_convoy(0, kc, xts0, vp0, psum_f, "f")

            def v1(kc):
                return lambda: emit_v_convoy(1, kc, xts1_box[0], vp1,
                                             psum_f, "f")

            def load_x1():
                xts1_box[0], xts1_box[1] = load_x(1)

            KOF = NCC  # k output-chunk offset
            fill0 = {
                (0, 0): [v0(3)],
                (0, 1): [load_x1],
                (0, 2): [v0(4)],
                (0, 3): [qk0(KOF + 1)],          # k1 (needed h=2)
                (0, 4): [v0(5)],
                (0, 5): [load_pw],
                (0, 6): [v0(6)],
                (1, 2): [qk0(2)],                # q2 (h=4)
                (1, 5): [qk0(KOF + 2)],          # k2
                (2, 2): [qk0(3)],                # q3 (h=6)
                (2, 5): [qk0(KOF + 3)],          # k3
                (3, 2): [qk0(4)],                # q4 (h=8)
                (3, 5): [qk0(KOF + 4)],          # k4
                (4, 2): [qk0(5)],                # q5 (h=10)
                (4, 5): [qk0(KOF + 5)],          # k5
                (5, 2): [qk1(0)],
                (5, 5): [qk1(KOF + 0)],
                (6, 2): [v1(0)],
                (7, 2): [v1(1)],
                (8, 2): [v1(2)],
                (9, 2): [v1(3)],
                (10, 2): [v1(4)],
                (10, 5): [v1(5)],
                (11, 2): [qk1(1)],               # needed attn1 h=2
                (11, 5): [qk1(KOF + 1)],
            }

            def proj0(tt):
                return lambda: emit_proj_convoy(0, tt, oT0, psum_f, "f")

            def n0(j):
                return lambda: emit_norm_pair(0, j, oT0)

            def n1(j):
                return lambda: emit_norm_pair(1, j, oT1)

            fill1 = {
                (0, 1): [v1(6)],
                (0, 2): [n0(0)],
                (0, 3): [qk1(2)],                # needed h=4
                (0, 4): [n0(1)],
                (0, 6): [qk1(KOF + 2)],
                (1, 1): [n0(2)],
                (1, 2): [qk1(3)],                # h=6
                (1, 4): [n0(3)],
                (1, 5): [qk1(KOF + 3)],
                (2, 1): [n0(4)],
                (2, 2): [qk1(4)],                # h=8
                (2, 4): [n0(5)],
                (2, 5): [qk1(KOF + 4)],
                (3, 2): [qk1(5)],                # h=10
                (3, 5): [qk1(KOF + 5)],
                (4, 2): [proj0(0)],
                (5, 1): [n1(0)],
                (5, 2): [proj0(1)],
                (6, 2): [proj0(2)],
                (7, 1): [n1(1)],
                (7, 2): [proj0(3)],
                (8, 2): [proj0(4)],
                (9, 1): [n1(2)],
                (9, 2): [proj0(5)],
                (10, 2): [proj0(6)],
                (11, 1): [n1(3)],
                (H, 0): [n1(4)],
            }

            emit_attention(0, qT0, kT0, vp0, oT0, fill0)
            emit_attention(1, qT1, kT1, vp1, oT1, fill1)

            # tail: last normalize pair's DMA chain hides under split proj
            # accumulation — contraction chunks 0-4 (pairs already normalized)
            # run across 4 open psum slots while pair 5's 1/d lands; chunk 5
            # joins in part2.
            emit_norm_pair(1, 5, oT1)
            tail_ps = {}
            tail_pool = [(psum_s, "s"), (psum_f, "f"),
                         (psum_o, "o"), (psum_s, "s")]
            for tt in range(4):
                pool, ptag = tail_pool[tt]
                tail_ps[tt] = emit_proj_part1(1, tt, oT1, pool, ptag, NCC - 1)
            for tt in range(4):
                emit_proj_part2(1, tt, oT1, tail_ps[tt], NCC - 1)
            for i, tt in enumerate(range(4, NKC)):
                pool, ptag = [(psum_f, "f"), (psum_o, "o"),
                              (psum_s, "s")][i % 3]
                emit_proj_convoy(1, tt, oT1, pool, ptag)

    nc.compile()
    return nc


def _relative_position_index():
    coords = np.stack(np.meshgrid(np.arange(WX), np.arange(WY), indexing="ij"))
    cf = coords.reshape(2, -1)
    rel = cf[:, :, None] - cf[:, None, :]
    rel = rel.transpose(1, 2, 0).astype(np.int64)
    rel[:, :, 0] += WX - 1
    rel[:, :, 1] += WY - 1
    rel[:, :, 0] *= 2 * WY - 1
    return rel.sum(-1)  # [L, L]


def _host_prep(x, qkv_w, proj_w, proj_b, rel_table, g2l, g2g):
    x = np.asarray(x, np.float32)
    qkv_w = np.asarray(qkv_w, np.float32)
    proj_w = np.asarray(proj_w, np.float32)
    proj_b = np.asarray(proj_b, np.float32)
    rel_table = np.asarray(rel_table, np.float32)
    g2l = np.asarray(g2l, np.float32)
    g2g = np.asarray(g2g, np.float32)

    bf16 = ml_dtypes.bfloat16
    f8 = ml_dtypes.float8_e4m3fn
    # x and q/k weights ship as fp8e4 in the [.., 128, 2, *] DoubleRow
    # layout: contraction chunk p covers C-rows [256p, 256p+256), subtile
    # s = rows [256p+128s, +128). Weight columns pre-scaled into fp8's
    # normal range; the psum evacuation rescales by 1/QS, 1/KS.
    xT = x.transpose(0, 2, 1)                                      # [B, C, N]
    xT16 = np.ascontiguousarray(xT).astype(bf16)
    x8 = np.clip(xT, -240, 240).astype(f8)
    x8 = np.ascontiguousarray(
        x8.reshape(B, 3, 2, 128, N).transpose(0, 1, 3, 2, 4))     # [B,3,128,2,N]
    qk_wT = qkv_w[:2 * C].T.copy()                                 # [C, 2C]
    qk_wT[:, :C] *= SCALE * QS
    qk_wT[:, C:] *= KS
    w8 = np.clip(qk_wT, -240, 240).astype(f8)
    # [C, 2C] -> [ccp 3, block 2, 128, 2, 768]
    w8 = np.ascontiguousarray(
        w8.reshape(3, 2, 128, 2, C).transpose(0, 3, 2, 1, 4))
    v_wT = np.ascontiguousarray(qkv_w[2 * C:].T).astype(bf16)      # [C, C]
    proj_wT = np.ascontiguousarray(proj_w.T).astype(bf16)          # [C, C]
    pb = proj_b.reshape(1, C).astype(bf16)

    # expB[h, k, q] = exp(bias[h, q, k]); exp applied at table granularity,
    # then expanded by the constant-index relative-position gather.
    ridx = _relative_position_index()
    et = np.exp(rel_table)                                         # [3025, H]
    eg2l = np.exp(g2l)                                             # [2, H, 1]
    eg2g = np.exp(g2g)                                             # [H, 1, 1]
    expB = np.empty((H, N, N), np.float32)
    expB[:, 1:, 1:] = et[ridx].transpose(2, 1, 0)                  # [H, k, q]
    expB[:, 0, 0] = eg2g[:, 0, 0]
    expB[:, 1:, 0] = eg2l[0][:, 0][None, :].T                      # global query
    expB[:, 0, 1:] = eg2l[1][:, 0][:, None]                        # global key
    expB16 = expB.astype(bf16)

    in_maps = []
    for i in range(N_CORES):
        in_maps.append({
            "xT": xT16[i * B_LOC:(i + 1) * B_LOC],
            "x8": x8[i * B_LOC:(i + 1) * B_LOC],
            "qk_w8": w8,
            "v_wT": v_wT,
            "proj_wT": proj_wT,
            "proj_b": pb,
            "expB": expB16,
        })
    return in_maps


_NC = None


def get_nc():
    global _NC
    if _NC is None:
        _NC = build_nc()
    return _NC


def kernel(x, qkv_w, proj_w, proj_b, rel_table, g2l, g2g):
    in_maps = _host_prep(x, qkv_w, proj_w, proj_b, rel_table, g2l, g2g)
    nc = get_nc()
    res = run_bass_kernel_spmd(nc, in_maps, core_ids=list(range(N_CORES)))
    out = np.concatenate([res.results[i]["out"] for i in range(N_CORES)], axis=0)
    return out.astype(np.float32)


# revision 53
# speedup vs baseline: 1.2357x; 1.0239x over previous
"""Trainium2 Bass kernel for windowed/global sparse attention (Swin-style
relative-position bias + 1 global token), data-parallel over batch on 8 cores.

Shapes: B=16, N=785 (1 global + 28x28 local), C=768, H=12 heads, d=64.

Per-core device program (2 batches/core). Design notes:
  - qT/kT computed transposed ([d, tokens]) so S^T = K @ Q^T needs no
    transposes; v computed natural ([tokens, d]) with a ones column appended
    per head so the P @ V matmul also yields softmax denominators.
  - softmax: exp(S + bias) = exp(S) * expB with expB = exp(bias) gathered on
    host and shipped bf16; exp on ScalarE, multiply on VectorE (bf16 2x) with
    a fraction offloaded to GpSimd.
  - head-streamed schedule: for each (head h, key-chunk kc) step the TensorE
    stream carries S(h, kc) immediately followed by O(h-1, kc) — the dense
    O convoy rides inside the exp-paced S phase so the PE array never idles
    long enough for the HAM activity monitor to re-throttle the clock to
    1.2 GHz (43% of the old kernel's span ran cold).
  - PSUM discipline (8 banks): 2 rotating S slots + 1 O-convoy slot
    (allocated at first write) + 1 filler slot for qkv/v/proj convoys of the
    other/previous batch, which are spread between steps at (h, kc)
    granularity so no two convoys contend for the filler slot back-to-back.
  - denominators: O psum row 64 DMA'd straight to DRAM; reciprocal runs on
    a [128, 75]-reshaped view (0.5us instead of 5us at [12, 786]); 1/d is
    DMA-broadcast back (bf16) and multiplied into oT in place.
  - proj: bias applied via a ones-row matmul into the same psum accumulation,
    psum DMA'd straight to DRAM (no DVE add / evac).
"""

import numpy as np
import ml_dtypes

import concourse.bass as bass
import concourse.bacc as bacc
import concourse.tile as tile
from concourse import mybir
from concourse.bass_utils import run_bass_kernel_spmd

F32 = mybir.dt.float32
BF16 = mybir.dt.bfloat16
F8 = mybir.dt.float8e4
DR = mybir.MatmulPerfMode.DoubleRow

QS = 512.0   # host scale folded into q weight columns (with SCALE)
KS = 64.0    # host scale folded into k weight columns
VS = 64.0    # host scale folded into v weight columns

WX = WY = 28
NGLO = 1
H = 12
L = WX * WY            # 784
N = NGLO + L           # 785
C = 768
HD = C // H            # 64
SCALE = HD ** -0.5
B = 16
N_CORES = 8
B_LOC = B // N_CORES   # 2
NCC = C // 128         # 6 contraction chunks
NKC = (N + 127) // 128  # 7 key/token chunks (last = 17 rows)
W = 786                # padded free width for N-sized tiles (even)
DSTRIDE = 800          # flat stride for denominator rows in DRAM scratch
DPAD = 9600            # 12*800 = 128*75 for the reshaped reciprocal

CG_N = [(0, 512), (512, 274)]
CG_C = [(0, 512), (512, 256)]


def _kr(kc):
    return min(128, N - kc * 128)


def build_nc():
    nc = bacc.Bacc(None, target_bir_lowering=False)

    xT_d = nc.dram_tensor("xT", [B_LOC, C, N], BF16, kind="ExternalInput")
    x8_d = nc.dram_tensor("x8", [B_LOC, 3, 128, 2, N], F8, kind="ExternalInput")
    qkw8_d = nc.dram_tensor("qk_w8", [3, 2, 128, 2, C], F8,
                            kind="ExternalInput")
    vwT_d = nc.dram_tensor("v_wT", [C, C], BF16, kind="ExternalInput")
    pwT_d = nc.dram_tensor("proj_wT", [C, C], BF16, kind="ExternalInput")
    pb_d = nc.dram_tensor("proj_b", [1, C], BF16, kind="ExternalInput")
    expB_d = nc.dram_tensor("expB", [H, N, N], BF16, kind="ExternalInput")
    out_d = nc.dram_tensor("out", [B_LOC, N, C], F32, kind="ExternalOutput")
    dall_d = nc.dram_tensor("dall_scratch", [B_LOC, DPAD], F32)
    dinv_d = nc.dram_tensor("dinv_scratch", [B_LOC, DPAD], BF16)

    with tile.TileContext(nc) as tc:
        with (
            tc.tile_pool(name="consts", bufs=1) as consts,
            tc.tile_pool(name="perb", bufs=2) as perb,
            tc.tile_pool(name="expbp", bufs=8) as expbp,
            tc.tile_pool(name="flow", bufs=8) as flow,
            tc.tile_pool(name="ptp", bufs=16) as ptp,
            tc.tile_pool(name="norm", bufs=2) as normp,
            tc.tile_pool(name="outp", bufs=2) as outp,
            tc.tile_pool(name="psum_s", bufs=2, space=bass.MemorySpace.PSUM) as psum_s,
            tc.tile_pool(name="psum_o", bufs=1, space=bass.MemorySpace.PSUM) as psum_o,
            tc.tile_pool(name="psum_f", bufs=1, space=bass.MemorySpace.PSUM) as psum_f,
        ):
            # ---- resident weights ----
            # q/k weights as fp8 DoubleRow tiles, per (contraction-pair p,
            # block t in {q,k}): [128, 2, 768] — small pair-strides keep the
            # DoubleRow LDW AP legal. v weights stay bf16 (v-path noise
            # passes straight to the output; q/k noise is softmax-damped).
            qkw8 = [[consts.tile([128, 2, C], F8, tag=f"qkw{p}_{t}",
                                 name=f"qkw{p}_{t}") for t in range(2)]
                    for p in range(3)]
            vw16 = [consts.tile([128, C], BF16, tag=f"vw{cc}", name=f"vw{cc}")
                    for cc in range(NCC)]
            pw16 = [consts.tile([128, C], BF16, tag=f"pw{cc}", name=f"pw{cc}")
                    for cc in range(NCC)]
            pb16 = consts.tile([1, C], BF16, tag="pb16")
            ones = consts.tile([1, 128], BF16, tag="ones")
            nc.vector.memset(ones[:], 1.0)

            def load_qkvw():
                # q tiles via scalar queue, k via gpsimd — both land in ~2us
                # so the first q/k convoys start immediately; v weights after.
                for p in range(3):
                    nc.scalar.dma_start(qkw8[p][0][:], qkw8_d[p, 0])
                    nc.gpsimd.dma_start(qkw8[p][1][:], qkw8_d[p, 1])
                for cc in range(NCC):
                    eng = nc.scalar if cc % 2 == 0 else nc.gpsimd
                    eng.dma_start(
                        vw16[cc][:], vwT_d[cc * 128:(cc + 1) * 128, :])

            def load_pw():
                for cc in range(NCC):
                    nc.sync.dma_start(
                        pw16[cc][:], pwT_d[cc * 128:(cc + 1) * 128, :])
                nc.sync.dma_start(pb16[:], pb_d[:])

            XW = 800  # x8 tile pair-stride: 16-byte aligned for DoubleRow

            def load_x(b):
                """fp8 pair tiles first (q/k convoys consume them first);
                bf16 x via the scalar queue for batch 0 (idle pre-attention)
                so the sync queue reaches the ebt prefetches quickly."""
                x8s = []
                for p in range(3):
                    t = perb.tile([128, 2, XW], F8, tag=f"x8t{p}",
                                  name=f"x8t{p}_{b}")
                    nc.sync.dma_start(t[:, :, 0:N], x8_d[b, p])
                    x8s.append(t)
                xts = []
                for cc in range(NCC):
                    t = perb.tile([128, W], BF16, tag=f"xt{cc}",
                                  name=f"xt{cc}_{b}")
                    nc.sync.dma_start(
                        t[:, 0:N], xT_d[b, cc * 128:(cc + 1) * 128, :])
                    xts.append(t)
                return xts, x8s

            def alloc_qkT(b):
                qT = [perb.tile([128, W], BF16, tag=f"qT{i}", name=f"qT{i}_{b}")
                      for i in range(NCC)]
                kT = [perb.tile([128, W], BF16, tag=f"kT{i}", name=f"kT{i}_{b}")
                      for i in range(NCC)]
                return qT, kT

            def emit_qkT_convoy(b, oc, x8s, qT, kT, pool, ptag):
                """one output chunk (128 cols of q or k), contraction over C
                via 3 fp8 DoubleRow matmuls per column group."""
                ps = pool.tile([128, W], F32, tag=ptag, name=f"psqk{oc}_{b}")
                blk, col = (0, oc * 128) if oc < NCC else (1, (oc - NCC) * 128)
                for p in range(3):
                    for (c0, cn) in CG_N:
                        nc.tensor.matmul(
                            ps[:, c0:c0 + cn],
                            qkw8[p][blk][:, :, col:col + 128],
                            x8s[p][:, :, c0:c0 + cn],
                            start=(p == 0), stop=(p == 2),
                            perf_mode=DR,
                        )
                dst = qT[oc] if oc < NCC else kT[oc - NCC]
                sc = 1.0 / QS if oc < NCC else 1.0 / KS
                nc.vector.tensor_scalar_mul(dst[:, 0:N], ps[:, 0:N], sc)

            def alloc_vp(b):
                return [perb.tile([128, H * (HD + 1)], BF16, tag=f"vp{i}",
                                  name=f"vp{i}_{b}") for i in range(NKC)]

            def emit_v_convoy(b, kc, xts, vp, pool, ptag):
                kr = _kr(kc)
                ps = pool.tile([128, W], F32, tag=ptag, name=f"psv{kc}_{b}")
                for cc in range(NCC):
                    for (c0, cn) in CG_C:
                        nc.tensor.matmul(
                            ps[0:kr, c0:c0 + cn],
                            xts[cc][:, kc * 128:kc * 128 + kr],
                            vw16[cc][:, c0:c0 + cn],
                            start=(cc == 0), stop=(cc == NCC - 1),
                        )
                v3 = vp[kc][:].rearrange("p (h e) -> p h e", e=HD + 1)
                nc.vector.tensor_copy(
                    v3[0:kr, :, 0:HD],
                    ps[0:kr, 0:C].rearrange("p (h d) -> p h d", d=HD),
                )
                nc.vector.memset(v3[0:kr, :, HD:HD + 1], 1.0)

            def emit_proj_part1(b, tt, oT, pool, ptag, ncc1):
                """bias + contraction chunks 0..ncc1-1, psum left open."""
                ts_ = _kr(tt)
                ps = pool.tile([128, W], F32, tag=ptag, name=f"psp{tt}_{b}")
                for (c0, cn) in CG_C:
                    nc.tensor.matmul(
                        ps[0:ts_, c0:c0 + cn],
                        ones[0:1, 0:ts_],
                        pb16[0:1, c0:c0 + cn],
                        start=True, stop=False,
                    )
                    for cc in range(ncc1):
                        nc.tensor.matmul(
                            ps[0:ts_, c0:c0 + cn],
                            oT[cc][:, tt * 128:tt * 128 + ts_],
                            pw16[cc][:, c0:c0 + cn],
                            start=False, stop=False,
                        )
                return ps

            def emit_proj_part2(b, tt, oT, ps, ncc1):
                ts_ = _kr(tt)
                for (c0, cn) in CG_C:
                    for cc in range(ncc1, NCC):
                        nc.tensor.matmul(
                            ps[0:ts_, c0:c0 + cn],
                            oT[cc][:, tt * 128:tt * 128 + ts_],
                            pw16[cc][:, c0:c0 + cn],
                            start=False, stop=(cc == NCC - 1),
                        )
                ob = outp.tile([128, C], F32, tag="ob", name=f"ob{tt}_{b}")
                nc.vector.tensor_copy(ob[0:ts_, :], ps[0:ts_, 0:C])
                nc.sync.dma_start(
                    out_d[b, tt * 128:tt * 128 + ts_, :], ob[0:ts_, :])

            def emit_proj_convoy(b, tt, oT, pool, ptag):
                """one token chunk of the projection, bias via ones-matmul."""
                ps = emit_proj_part1(b, tt, oT, pool, ptag, NCC - 1)
                emit_proj_part2(b, tt, oT, ps, NCC - 1)

            def emit_norm_pair(b, j, oT):
                """in-place oT[j] *= 1/d: per-pair reciprocal on a [64, 25]
                reshaped view of the pair's two denominator rows, then
                DMA-broadcast of 1/d."""
                base = 2 * j * DSTRIDE
                da = normp.tile([64, 25], F32, tag="da", name=f"da{j}_{b}")
                nc.sync.dma_start(
                    da[:], dall_d[b, base:base + 1600]
                    .rearrange("(p f) -> p f", f=25))
                di = normp.tile([64, 25], BF16, tag="di", name=f"di{j}_{b}")
                with nc.allow_low_precision(reason="1/d broadcast in bf16"):
                    nc.vector.reciprocal(di[:], da[:])
                nc.sync.dma_start(
                    dinv_d[b, base:base + 1600]
                    .rearrange("(p f) -> p f", f=25), di[:])
                dr = normp.tile([128, W], BF16, tag="drep", name=f"dr{j}_{b}")
                for hh in range(2):
                    row = dinv_d[b, (2 * j + hh) * DSTRIDE:
                                 (2 * j + hh) * DSTRIDE + N]
                    src = bass.AP(tensor=row.tensor, offset=row.offset,
                                  ap=[[0, 64]] + row.ap)
                    nc.sync.dma_start(dr[hh * 64:(hh + 1) * 64, 0:N], src)
                nc.vector.tensor_tensor(
                    oT[j][:, 0:N], oT[j][:, 0:N], dr[:, 0:N],
                    mybir.AluOpType.mult)

            # ---------------- attention ----------------
            def emit_attention(b, qT, kT, vp, oT, fillers):
                """head-streamed: per (h, kc) step the PE stream carries
                S(h, kc) then O(h-1, kc); filler closures attached to (h, kc)
                run after that step's emission. fillers[(h, kc)] -> [fn]."""
                steps = [(h, kc) for h in range(H) for kc in range(NKC)]
                pts = {}
                psO = {}

                def issue_ebt(idx):
                    h, kc = steps[idx]
                    kr = _kr(kc)
                    t = expbp.tile([128, W], BF16, tag="expb",
                                   name=f"ebt{h}_{kc}_{b}")
                    eng = nc.gpsimd if (idx % 3 == 2) else nc.sync
                    eng.dma_start(t[0:kr, 0:N],
                                  expB_d[h, kc * 128:kc * 128 + kr, :])
                    return t

                ebt_q = {}
                for i in range(5):
                    ebt_q[i] = issue_ebt(i)

                def emit_O_step(h, kc):
                    kr = _kr(kc)
                    if kc == 0:
                        psO[h] = psum_o.tile([HD + 1, W], F32, tag="o",
                                             name=f"pso{h}_{b}")
                    pt = pts.pop((h, kc))
                    for (c0, cn) in CG_N:
                        nc.tensor.matmul(
                            psO[h][:, c0:c0 + cn],
                            vp[kc][0:kr, h * (HD + 1):(h + 1) * (HD + 1)],
                            pt[0:kr, c0:c0 + cn],
                            start=(kc == 0), stop=(kc == NKC - 1),
                        )

                def emit_O_evac(h):
                    j, hh = h // 2, h % 2
                    nc.vector.tensor_copy(
                        oT[j][hh * 64:hh * 64 + 64, 0:N], psO[h][0:64, 0:N])
                    dn = normp.tile([65, W], F32, tag="dn", name=f"dn{h}_{b}")
                    nc.scalar.copy(dn[64:65, 0:W], psO[h][64:65, 0:W])
                    nc.sync.dma_start(
                        dall_d[b, h * DSTRIDE:h * DSTRIDE + W],
                        dn[64:65, 0:W])

                for i, (h, kc) in enumerate(steps):
                    j = h // 2
                    po = (h % 2) * 64
                    kr = _kr(kc)
                    # S matmuls
                    ps = psum_s.tile([128, W], F32, tag="s",
                                     name=f"pss{h}_{kc}_{b}")
                    for (c0, cn) in CG_N:
                        nc.tensor.matmul(
                            ps[0:kr, c0:c0 + cn],
                            kT[j][po:po + 64, kc * 128:kc * 128 + kr],
                            qT[j][po:po + 64, c0:c0 + cn],
                            start=True, stop=True,
                        )
                    # O for previous head rides in the same step
                    if h > 0:
                        emit_O_step(h - 1, kc)
                    # exp + expB multiply
                    es = flow.tile([128, W], BF16, tag="expS",
                                   name=f"es{h}_{kc}_{b}")
                    nc.scalar.activation(
                        es[0:kr, 0:W], ps[0:kr, 0:W],
                        mybir.ActivationFunctionType.Exp)
                    pt = ptp.tile([128, W], BF16, tag="pT",
                                  name=f"pt{h}_{kc}_{b}")
                    meng = nc.gpsimd if (i % 3 == 2) else nc.vector
                    meng.tensor_tensor(
                        pt[0:kr, 0:N], es[0:kr, 0:N], ebt_q.pop(i)[0:kr, 0:N],
                        mybir.AluOpType.mult)
                    pts[(h, kc)] = pt
                    if i + 5 < len(steps):
                        ebt_q[i + 5] = issue_ebt(i + 5)
                    # previous head's O evac at its boundary
                    if kc == NKC - 1 and h > 0:
                        emit_O_evac(h - 1)
                    for f in fillers.get((h, kc), []):
                        f()
                # trailing O convoy for the last head
                for kc in range(NKC):
                    emit_O_step(H - 1, kc)
                emit_O_evac(H - 1)
                for f in fillers.get((H, 0), []):
                    f()

            # ---------------- program ----------------
            xts0, x8s0 = load_x(0)
            load_qkvw()
            qT0, kT0 = alloc_qkT(0)
            qT1, kT1 = alloc_qkT(1)
            # head phase: only q0, k0, q1 before attention starts — V0
            # convoys ride as step fillers so S(0,0) issues immediately.
            emit_qkT_convoy(0, 0, x8s0, qT0, kT0, psum_s, "s")
            emit_qkT_convoy(0, NCC + 0, x8s0, qT0, kT0, psum_s, "s")
            emit_qkT_convoy(0, 1, x8s0, qT0, kT0, psum_s, "s")
            vp0 = alloc_vp(0)
            oT0 = [perb.tile([128, W], BF16, tag=f"oT{i}", name=f"oT{i}_0")
                   for i in range(NCC)]
            oT1 = [perb.tile([128, W], BF16, tag=f"oT{i}", name=f"oT{i}_1")
                   for i in range(NCC)]
            vp1 = alloc_vp(1)
            xts1_box = {}

            def qk0(oc):
                return lambda: emit_qkT_convoy(0, oc, x8s0, qT0, kT0,
                                               psum_f, "f")

            def qk1(oc):
                return lambda: emit_qkT_convoy(1, oc, xts1_box[1], qT1, kT1,
                                               psum_f, "f")

            def v0(kc, pool, ptag):
                return lambda: emit_v_convoy(0, kc, xts0, vp0, pool, ptag)

            def v1(kc):
                return lambda: emit_v_convoy(1, kc, xts1_box[0], vp1,
                                             psum_f, "f")

            def load_x1():
                xts1_box[0], xts1_box[1] = load_x(1)

            KOF = NCC  # k output-chunk offset
            fill0 = {
                (0, 1): [v0(0, psum_o, "o")],
                (0, 2): [v0(1, psum_f, "f"), load_x1],
                (0, 3): [v0(2, psum_o, "o")],
                (0, 4): [v0(3, psum_f, "f"), load_pw],
                (0, 5): [v0(4, psum_o, "o")],
                (0, 6): [v0(5, psum_f, "f")],
                (1, 1): [v0(6, psum_f, "f")],
                (1, 3): [qk0(KOF + 1)],          # k1 (needed h=2)
                (1, 6): [qk0(2)],                # q2 (h=4)
                (2, 2): [qk0(KOF + 2)],          # k2
                (2, 5): [qk0(3)],                # q3 (h=6)
                (3, 2): [qk0(KOF + 3)],          # k3
                (3, 5): [qk0(4)],                # q4 (h=8)
                (4, 2): [qk0(KOF + 4)],          # k4
                (4, 5): [qk0(5)],                # q5 (h=10)
                (5, 2): [qk0(KOF + 5)],          # k5
                (5, 5): [qk1(0)],
                (6, 2): [qk1(KOF + 0)],
                (6, 5): [v1(0)],
                (7, 2): [v1(1)],
                (8, 2): [v1(2)],
                (9, 2): [v1(3)],
                (10, 2): [v1(4)],
                (10, 5): [v1(5)],
                (11, 2): [qk1(1)],               # needed attn1 h=2
                (11, 5): [qk1(KOF + 1)],
            }

            def proj0(tt):
                return lambda: emit_proj_convoy(0, tt, oT0, psum_f, "f")

            def n0(j):
                return lambda: emit_norm_pair(0, j, oT0)

            def n1(j):
                return lambda: emit_norm_pair(1, j, oT1)

            fill1 = {
                (0, 1): [v1(6)],
                (0, 2): [n0(0)],
                (0, 3): [qk1(2)],                # needed h=4
                (0, 4): [n0(1)],
                (0, 6): [qk1(KOF + 2)],
                (1, 1): [n0(2)],
                (1, 2): [qk1(3)],                # h=6
                (1, 4): [n0(3)],
                (1, 5): [qk1(KOF + 3)],
                (2, 1): [n0(4)],
                (2, 2): [qk1(4)],                # h=8
                (2, 4): [n0(5)],
                (2, 5): [qk1(KOF + 4)],
                (3, 2): [qk1(5)],                # h=10
                (3, 5): [qk1(KOF + 5)],
                (4, 2): [proj0(0)],
                (5, 1): [n1(0)],
                (5, 2): [proj0(1)],
                (6, 2): [proj0(2)],
                (7, 1): [n1(1)],
                (7, 2): [proj0(3)],
                (8, 2): [proj0(4)],
                (9, 1): [n1(2)],
                (9, 2): [proj0(5)],
                (10, 2): [proj0(6)],
                (11, 1): [n1(3)],
                (H, 0): [n1(4)],
            }

            emit_attention(0, qT0, kT0, vp0, oT0, fill0)
            emit_attention(1, qT1, kT1, vp1, oT1, fill1)

            # tail: last normalize pair's DMA chain hides under split proj
            # accumulation — contraction chunks 0-4 (pairs already normalized)
            # run across 4 open psum slots while pair 5's 1/d lands; chunk 5
            # joins in part2.
            emit_norm_pair(1, 5, oT1)
            tail_ps = {}
            tail_pool = [(psum_s, "s"), (psum_f, "f"),
                         (psum_o, "o"), (psum_s, "s")]
            for tt in range(4):
                pool, ptag = tail_pool[tt]
                tail_ps[tt] = emit_proj_part1(1, tt, oT1, pool, ptag, NCC - 1)
            for tt in range(4):
                emit_proj_part2(1, tt, oT1, tail_ps[tt], NCC - 1)
            for i, tt in enumerate(range(4, NKC)):
                pool, ptag = [(psum_f, "f"), (psum_o, "o"),
                              (psum_s, "s")][i % 3]
                emit_proj_convoy(1, tt, oT1, pool, ptag)

    nc.compile()
    return nc


def _relative_position_index():
    coords = np.stack(np.meshgrid(np.arange(WX), np.arange(WY), indexing="ij"))
    cf = coords.reshape(2, -1)
    rel = cf[:, :, None] - cf[:, None, :]
    rel = rel.transpose(1, 2, 0).astype(np.int64)
    rel[:, :, 0] += WX - 1
    rel[:, :, 1] += WY - 1
    rel[:, :, 0] *= 2 * WY - 1
    return rel.sum(-1)  # [L, L]


def _host_prep(x, qkv_w, proj_w, proj_b, rel_table, g2l, g2g):
    x = np.asarray(x, np.float32)
    qkv_w = np.asarray(qkv_w, np.float32)
    proj_w = np.asarray(proj_w, np.float32)
    proj_b = np.asarray(proj_b, np.float32)
    rel_table = np.asarray(rel_table, np.float32)
    g2l = np.asarray(g2l, np.float32)
    g2g = np.asarray(g2g, np.float32)

    bf16 = ml_dtypes.bfloat16
    f8 = ml_dtypes.float8_e4m3fn
    # x and q/k weights ship as fp8e4 in the [.., 128, 2, *] DoubleRow
    # layout: contraction chunk p covers C-rows [256p, 256p+256), subtile
    # s = rows [256p+128s, +128). Weight columns pre-scaled into fp8's
    # normal range; the psum evacuation rescales by 1/QS, 1/KS.
    xT = x.transpose(0, 2, 1)                                      # [B, C, N]
    xT16 = np.ascontiguousarray(xT).astype(bf16)
    x8 = np.clip(xT, -240, 240).astype(f8)
    x8 = np.ascontiguousarray(
        x8.reshape(B, 3, 2, 128, N).transpose(0, 1, 3, 2, 4))     # [B,3,128,2,N]
    qk_wT = qkv_w[:2 * C].T.copy()                                 # [C, 2C]
    qk_wT[:, :C] *= SCALE * QS
    qk_wT[:, C:] *= KS
    w8 = np.clip(qk_wT, -240, 240).astype(f8)
    # [C, 2C] -> [ccp 3, block 2, 128, 2, 768]
    w8 = np.ascontiguousarray(
        w8.reshape(3, 2, 128, 2, C).transpose(0, 3, 2, 1, 4))
    v_wT = np.ascontiguousarray(qkv_w[2 * C:].T).astype(bf16)      # [C, C]
    proj_wT = np.ascontiguousarray(proj_w.T).astype(bf16)          # [C, C]
    pb = proj_b.reshape(1, C).astype(bf16)

    # expB[h, k, q] = exp(bias[h, q, k]); exp applied at table granularity,
    # then expanded by the constant-index relative-position gather.
    ridx = _relative_position_index()
    et = np.exp(rel_table)                                         # [3025, H]
    eg2l = np.exp(g2l)                                             # [2, H, 1]
    eg2g = np.exp(g2g)                                             # [H, 1, 1]
    expB = np.empty((H, N, N), np.float32)
    expB[:, 1:, 1:] = et[ridx].transpose(2, 1, 0)                  # [H, k, q]
    expB[:, 0, 0] = eg2g[:, 0, 0]
    expB[:, 1:, 0] = eg2l[0][:, 0][None, :].T                      # global query
    expB[:, 0, 1:] = eg2l[1][:, 0][:, None]                        # global key
    expB16 = expB.astype(bf16)

    in_maps = []
    for i in range(N_CORES):
        in_maps.append({
            "xT": xT16[i * B_LOC:(i + 1) * B_LOC],
            "x8": x8[i * B_LOC:(i + 1) * B_LOC],
            "qk_w8": w8,
            "v_wT": v_wT,
            "proj_wT": proj_wT,
            "proj_b": pb,
            "expB": expB16,
        })
    return in_maps


_NC = None


def get_nc():
    global _NC
    if _NC is None:
        _NC = build_nc()
    return _NC


def kernel(x, qkv_w, proj_w, proj_b, rel_table, g2l, g2g):
    in_maps = _host_prep(x, qkv_w, proj_w, proj_b, rel_table, g2l, g2g)
    nc = get_nc()
    res = run_bass_kernel_spmd(nc, in_maps, core_ids=list(range(N_CORES)))
    out = np.concatenate([res.results[i]["out"] for i in range(N_CORES)], axis=0)
    return out.astype(np.float32)


# revision 57
# speedup vs baseline: 1.2458x; 1.0082x over previous
"""Trainium2 Bass kernel for windowed/global sparse attention (Swin-style
relative-position bias + 1 global token), data-parallel over batch on 8 cores.

Shapes: B=16, N=785 (1 global + 28x28 local), C=768, H=12 heads, d=64.

Per-core device program (2 batches/core). Design notes:
  - qT/kT computed transposed ([d, tokens]) so S^T = K @ Q^T needs no
    transposes; v computed natural ([tokens, d]) with a ones column appended
    per head so the P @ V matmul also yields softmax denominators.
  - softmax: exp(S + bias) = exp(S) * expB with expB = exp(bias) gathered on
    host and shipped bf16; exp on ScalarE, multiply on VectorE (bf16 2x) with
    a fraction offloaded to GpSimd.
  - head-streamed schedule: for each (head h, key-chunk kc) step the TensorE
    stream carries S(h, kc) immediately followed by O(h-1, kc) — the dense
    O convoy rides inside the exp-paced S phase so the PE array never idles
    long enough for the HAM activity monitor to re-throttle the clock to
    1.2 GHz (43% of the old kernel's span ran cold).
  - PSUM discipline (8 banks): 2 rotating S slots + 1 O-convoy slot
    (allocated at first write) + 1 filler slot for qkv/v/proj convoys of the
    other/previous batch, which are spread between steps at (h, kc)
    granularity so no two convoys contend for the filler slot back-to-back.
  - denominators: O psum row 64 DMA'd straight to DRAM; reciprocal runs on
    a [128, 75]-reshaped view (0.5us instead of 5us at [12, 786]); 1/d is
    DMA-broadcast back (bf16) and multiplied into oT in place.
  - proj: bias applied via a ones-row matmul into the same psum accumulation,
    psum DMA'd straight to DRAM (no DVE add / evac).
"""

import numpy as np
import ml_dtypes

import concourse.bass as bass
import concourse.bacc as bacc
import concourse.tile as tile
from concourse import mybir
from concourse.bass_utils import run_bass_kernel_spmd

F32 = mybir.dt.float32
BF16 = mybir.dt.bfloat16
F8 = mybir.dt.float8e4
DR = mybir.MatmulPerfMode.DoubleRow

QS = 512.0   # host scale folded into q weight columns (with SCALE)
KS = 64.0    # host scale folded into k weight columns
VS = 64.0    # host scale folded into v weight columns

WX = WY = 28
NGLO = 1
H = 12
L = WX * WY            # 784
N = NGLO + L           # 785
C = 768
HD = C // H            # 64
SCALE = HD ** -0.5
B = 16
N_CORES = 8
B_LOC = B // N_CORES   # 2
NCC = C // 128         # 6 contraction chunks
NKC = (N + 127) // 128  # 7 key/token chunks (last = 17 rows)
W = 786                # padded free width for N-sized tiles (even)
DSTRIDE = 800          # flat stride for denominator rows in DRAM scratch
DPAD = 9600            # 12*800 = 128*75 for the reshaped reciprocal

CG_N = [(0, 512), (512, 274)]
CG_C = [(0, 512), (512, 256)]


def _kr(kc):
    return min(128, N - kc * 128)


def build_nc():
    nc = bacc.Bacc(None, target_bir_lowering=False)

    xT_d = nc.dram_tensor("xT", [B_LOC, C, N], BF16, kind="ExternalInput")
    x8_d = nc.dram_tensor("x8", [B_LOC, 3, 128, 2, N], F8, kind="ExternalInput")
    qkw8_d = nc.dram_tensor("qk_w8", [3, 2, 128, 2, C], F8,
                            kind="ExternalInput")
    vwT_d = nc.dram_tensor("v_wT", [C, C], BF16, kind="ExternalInput")
    pwT_d = nc.dram_tensor("proj_wT", [C, C], BF16, kind="ExternalInput")
    pb_d = nc.dram_tensor("proj_b", [1, C], BF16, kind="ExternalInput")
    expB_d = nc.dram_tensor("expB", [H, N, N], BF16, kind="ExternalInput")
    out_d = nc.dram_tensor("out", [B_LOC, N, C], F32, kind="ExternalOutput")
    dall_d = nc.dram_tensor("dall_scratch", [B_LOC, DPAD], F32)
    dinv_d = nc.dram_tensor("dinv_scratch", [B_LOC, DPAD], BF16)

    with tile.TileContext(nc) as tc:
        with (
            tc.tile_pool(name="consts", bufs=1) as consts,
            tc.tile_pool(name="perb", bufs=2) as perb,
            tc.tile_pool(name="expbp", bufs=8) as expbp,
            tc.tile_pool(name="flow", bufs=8) as flow,
            tc.tile_pool(name="ptp", bufs=16) as ptp,
            tc.tile_pool(name="norm", bufs=2) as normp,
            tc.tile_pool(name="outp", bufs=2) as outp,
            tc.tile_pool(name="psum_s", bufs=2, space=bass.MemorySpace.PSUM) as psum_s,
            tc.tile_pool(name="psum_o", bufs=1, space=bass.MemorySpace.PSUM) as psum_o,
            tc.tile_pool(name="psum_f", bufs=1, space=bass.MemorySpace.PSUM) as psum_f,
        ):
            # ---- resident weights ----
            # q/k weights as fp8 DoubleRow tiles, per (contraction-pair p,
            # block t in {q,k}): [128, 2, 768] — small pair-strides keep the
            # DoubleRow LDW AP legal. v weights stay bf16 (v-path noise
            # passes straight to the output; q/k noise is softmax-damped).
            qkw8 = [[consts.tile([128, 2, C], F8, tag=f"qkw{p}_{t}",
                                 name=f"qkw{p}_{t}") for t in range(2)]
                    for p in range(3)]
            vw16 = [consts.tile([128, C], BF16, tag=f"vw{cc}", name=f"vw{cc}")
                    for cc in range(NCC)]
            pw16 = [consts.tile([128, C], BF16, tag=f"pw{cc}", name=f"pw{cc}")
                    for cc in range(NCC)]
            pb16 = consts.tile([1, C], BF16, tag="pb16")
            ones = consts.tile([1, 128], BF16, tag="ones")
            nc.vector.memset(ones[:], 1.0)

            def load_qkvw():
                # q tiles via scalar queue, k via gpsimd — both land in ~2us
                # so the first q/k convoys start immediately; v weights after.
                for p in range(3):
                    nc.scalar.dma_start(qkw8[p][0][:], qkw8_d[p, 0])
                    nc.gpsimd.dma_start(qkw8[p][1][:], qkw8_d[p, 1])
                for cc in range(NCC):
                    eng = nc.scalar if cc % 2 == 0 else nc.gpsimd
                    eng.dma_start(
                        vw16[cc][:], vwT_d[cc * 128:(cc + 1) * 128, :])

            def load_pw():
                for cc in range(NCC):
                    nc.sync.dma_start(
                        pw16[cc][:], pwT_d[cc * 128:(cc + 1) * 128, :])
                nc.sync.dma_start(pb16[:], pb_d[:])

            XW = 800  # x8 tile pair-stride: 16-byte aligned for DoubleRow

            def load_x(b):
                """fp8 pair tiles first (q/k convoys consume them first);
                bf16 x via the scalar queue for batch 0 (idle pre-attention)
                so the sync queue reaches the ebt prefetches quickly."""
                x8s = []
                for p in range(3):
                    t = perb.tile([128, 2, XW], F8, tag=f"x8t{p}",
                                  name=f"x8t{p}_{b}")
                    nc.sync.dma_start(t[:, :, 0:N], x8_d[b, p])
                    x8s.append(t)
                xts = []
                for cc in range(NCC):
                    t = perb.tile([128, W], BF16, tag=f"xt{cc}",
                                  name=f"xt{cc}_{b}")
                    nc.sync.dma_start(
                        t[:, 0:N], xT_d[b, cc * 128:(cc + 1) * 128, :])
                    xts.append(t)
                return xts, x8s

            def alloc_qkT(b):
                qT = [perb.tile([128, W], BF16, tag=f"qT{i}", name=f"qT{i}_{b}")
                      for i in range(NCC)]
                kT = [perb.tile([128, W], BF16, tag=f"kT{i}", name=f"kT{i}_{b}")
                      for i in range(NCC)]
                return qT, kT

            def emit_qkT_convoy(b, oc, x8s, qT, kT, pool, ptag):
                """one output chunk (128 cols of q or k), contraction over C
                via 3 fp8 DoubleRow matmuls per column group."""
                ps = pool.tile([128, W], F32, tag=ptag, name=f"psqk{oc}_{b}")
                blk, col = (0, oc * 128) if oc < NCC else (1, (oc - NCC) * 128)
                for p in range(3):
                    for (c0, cn) in CG_N:
                        nc.tensor.matmul(
                            ps[:, c0:c0 + cn],
                            qkw8[p][blk][:, :, col:col + 128],
                            x8s[p][:, :, c0:c0 + cn],
                            start=(p == 0), stop=(p == 2),
                            perf_mode=DR,
                        )
                dst = qT[oc] if oc < NCC else kT[oc - NCC]
                sc = 1.0 / QS if oc < NCC else 1.0 / KS
                nc.vector.tensor_scalar_mul(dst[:, 0:N], ps[:, 0:N], sc)

            def alloc_vp(b):
                return [perb.tile([128, H * (HD + 1)], BF16, tag=f"vp{i}",
                                  name=f"vp{i}_{b}") for i in range(NKC)]

            def emit_v_convoy(b, kc, xts, vp, pool, ptag):
                kr = _kr(kc)
                ps = pool.tile([128, W], F32, tag=ptag, name=f"psv{kc}_{b}")
                for cc in range(NCC):
                    for (c0, cn) in CG_C:
                        nc.tensor.matmul(
                            ps[0:kr, c0:c0 + cn],
                            xts[cc][:, kc * 128:kc * 128 + kr],
                            vw16[cc][:, c0:c0 + cn],
                            start=(cc == 0), stop=(cc == NCC - 1),
                        )
                v3 = vp[kc][:].rearrange("p (h e) -> p h e", e=HD + 1)
                nc.vector.tensor_copy(
                    v3[0:kr, :, 0:HD],
                    ps[0:kr, 0:C].rearrange("p (h d) -> p h d", d=HD),
                )
                nc.vector.memset(v3[0:kr, :, HD:HD + 1], 1.0)

            def emit_proj_part1(b, tt, oT, pool, ptag, ncc1):
                """bias + contraction chunks 0..ncc1-1, psum left open."""
                ts_ = _kr(tt)
                ps = pool.tile([128, W], F32, tag=ptag, name=f"psp{tt}_{b}")
                for (c0, cn) in CG_C:
                    nc.tensor.matmul(
                        ps[0:ts_, c0:c0 + cn],
                        ones[0:1, 0:ts_],
                        pb16[0:1, c0:c0 + cn],
                        start=True, stop=False,
                    )
                    for cc in range(ncc1):
                        nc.tensor.matmul(
                            ps[0:ts_, c0:c0 + cn],
                            oT[cc][:, tt * 128:tt * 128 + ts_],
                            pw16[cc][:, c0:c0 + cn],
                            start=False, stop=False,
                        )
                return ps

            def emit_proj_part2(b, tt, oT, ps, ncc1):
                ts_ = _kr(tt)
                for (c0, cn) in CG_C:
                    for cc in range(ncc1, NCC):
                        nc.tensor.matmul(
                            ps[0:ts_, c0:c0 + cn],
                            oT[cc][:, tt * 128:tt * 128 + ts_],
                            pw16[cc][:, c0:c0 + cn],
                            start=False, stop=(cc == NCC - 1),
                        )
                ob = outp.tile([128, C], F32, tag="ob", name=f"ob{tt}_{b}")
                nc.vector.tensor_copy(ob[0:ts_, :], ps[0:ts_, 0:C])
                nc.sync.dma_start(
                    out_d[b, tt * 128:tt * 128 + ts_, :], ob[0:ts_, :])

            def emit_proj_convoy(b, tt, oT, pool, ptag):
                """one token chunk of the projection, bias via ones-matmul."""
                ps = emit_proj_part1(b, tt, oT, pool, ptag, NCC - 1)
                emit_proj_part2(b, tt, oT, ps, NCC - 1)

            def emit_norm_pair(b, j, oT, deng=None):
                """in-place oT[j] *= 1/d: per-pair reciprocal on a [64, 25]
                reshaped view of the pair's two denominator rows, then
                DMA-broadcast of 1/d. deng picks the DMA issue queue (the
                scalar queue is idle at the tail — avoids sync backlog)."""
                deng = deng or nc.sync
                base = 2 * j * DSTRIDE
                da = normp.tile([64, 25], F32, tag="da", name=f"da{j}_{b}")
                deng.dma_start(
                    da[:], dall_d[b, base:base + 1600]
                    .rearrange("(p f) -> p f", f=25))
                di = normp.tile([64, 25], BF16, tag="di", name=f"di{j}_{b}")
                with nc.allow_low_precision(reason="1/d broadcast in bf16"):
                    nc.vector.reciprocal(di[:], da[:])
                deng.dma_start(
                    dinv_d[b, base:base + 1600]
                    .rearrange("(p f) -> p f", f=25), di[:])
                dr = normp.tile([128, W], BF16, tag="drep", name=f"dr{j}_{b}")
                for hh in range(2):
                    row = dinv_d[b, (2 * j + hh) * DSTRIDE:
                                 (2 * j + hh) * DSTRIDE + N]
                    src = bass.AP(tensor=row.tensor, offset=row.offset,
                                  ap=[[0, 64]] + row.ap)
                    deng.dma_start(dr[hh * 64:(hh + 1) * 64, 0:N], src)
                nc.vector.tensor_tensor(
                    oT[j][:, 0:N], oT[j][:, 0:N], dr[:, 0:N],
                    mybir.AluOpType.mult)

            # ---------------- attention ----------------
            def emit_attention(b, qT, kT, vp, oT, fillers):
                """head-streamed: per (h, kc) step the PE stream carries
                S(h, kc) then O(h-1, kc); filler closures attached to (h, kc)
                run after that step's emission. fillers[(h, kc)] -> [fn]."""
                steps = [(h, kc) for h in range(H) for kc in range(NKC)]
                pts = {}
                psO = {}

                def issue_ebt(idx):
                    h, kc = steps[idx]
                    kr = _kr(kc)
                    t = expbp.tile([128, W], BF16, tag="expb",
                                   name=f"ebt{h}_{kc}_{b}")
                    eng = nc.gpsimd if (idx % 3 == 2) else nc.sync
                    eng.dma_start(t[0:kr, 0:N],
                                  expB_d[h, kc * 128:kc * 128 + kr, :])
                    return t

                ebt_q = {}
                for i in range(5):
                    ebt_q[i] = issue_ebt(i)

                def emit_O_step(h, kc):
                    kr = _kr(kc)
                    if kc == 0:
                        psO[h] = psum_o.tile([HD + 1, W], F32, tag="o",
                                             name=f"pso{h}_{b}")
                    pt = pts.pop((h, kc))
                    for (c0, cn) in CG_N:
                        nc.tensor.matmul(
                            psO[h][:, c0:c0 + cn],
                            vp[kc][0:kr, h * (HD + 1):(h + 1) * (HD + 1)],
                            pt[0:kr, c0:c0 + cn],
                            start=(kc == 0), stop=(kc == NKC - 1),
                        )

                def emit_O_evac(h):
                    j, hh = h // 2, h % 2
                    nc.vector.tensor_copy(
                        oT[j][hh * 64:hh * 64 + 64, 0:N], psO[h][0:64, 0:N])
                    dn = normp.tile([65, W], F32, tag="dn", name=f"dn{h}_{b}")
                    nc.scalar.copy(dn[64:65, 0:W], psO[h][64:65, 0:W])
                    deng = nc.scalar if (b == 1 and h >= 10) else nc.sync
                    deng.dma_start(
                        dall_d[b, h * DSTRIDE:h * DSTRIDE + W],
                        dn[64:65, 0:W])

                for i, (h, kc) in enumerate(steps):
                    j = h // 2
                    po = (h % 2) * 64
                    kr = _kr(kc)
                    # S matmuls
                    ps = psum_s.tile([128, W], F32, tag="s",
                                     name=f"pss{h}_{kc}_{b}")
                    for (c0, cn) in CG_N:
                        nc.tensor.matmul(
                            ps[0:kr, c0:c0 + cn],
                            kT[j][po:po + 64, kc * 128:kc * 128 + kr],
                            qT[j][po:po + 64, c0:c0 + cn],
                            start=True, stop=True,
                        )
                    # O for previous head rides in the same step
                    if h > 0:
                        emit_O_step(h - 1, kc)
                    # exp + expB multiply
                    es = flow.tile([128, W], BF16, tag="expS",
                                   name=f"es{h}_{kc}_{b}")
                    nc.scalar.activation(
                        es[0:kr, 0:W], ps[0:kr, 0:W],
                        mybir.ActivationFunctionType.Exp)
                    pt = ptp.tile([128, W], BF16, tag="pT",
                                  name=f"pt{h}_{kc}_{b}")
                    meng = nc.gpsimd if (i % 3 == 2) else nc.vector
                    meng.tensor_tensor(
                        pt[0:kr, 0:N], es[0:kr, 0:N], ebt_q.pop(i)[0:kr, 0:N],
                        mybir.AluOpType.mult)
                    pts[(h, kc)] = pt
                    if i + 5 < len(steps):
                        ebt_q[i + 5] = issue_ebt(i + 5)
                    # previous head's O evac at its boundary
                    if kc == NKC - 1 and h > 0:
                        emit_O_evac(h - 1)
                    for f in fillers.get((h, kc), []):
                        f()
                # trailing O convoy for the last head
                for kc in range(NKC):
                    emit_O_step(H - 1, kc)
                emit_O_evac(H - 1)
                for f in fillers.get((H, 0), []):
                    f()

            # ---------------- program ----------------
            xts0, x8s0 = load_x(0)
            load_qkvw()
            qT0, kT0 = alloc_qkT(0)
            qT1, kT1 = alloc_qkT(1)
            # head phase: only q0, k0, q1 before attention starts — V0
            # convoys ride as step fillers so S(0,0) issues immediately.
            emit_qkT_convoy(0, 0, x8s0, qT0, kT0, psum_s, "s")
            emit_qkT_convoy(0, NCC + 0, x8s0, qT0, kT0, psum_s, "s")
            emit_qkT_convoy(0, 1, x8s0, qT0, kT0, psum_s, "s")
            vp0 = alloc_vp(0)
            oT0 = [perb.tile([128, W], BF16, tag=f"oT{i}", name=f"oT{i}_0")
                   for i in range(NCC)]
            oT1 = [perb.tile([128, W], BF16, tag=f"oT{i}", name=f"oT{i}_1")
                   for i in range(NCC)]
            vp1 = alloc_vp(1)
            xts1_box = {}

            def qk0(oc):
                return lambda: emit_qkT_convoy(0, oc, x8s0, qT0, kT0,
                                               psum_f, "f")

            def qk1(oc):
                return lambda: emit_qkT_convoy(1, oc, xts1_box[1], qT1, kT1,
                                               psum_f, "f")

            def v0(kc, pool, ptag):
                return lambda: emit_v_convoy(0, kc, xts0, vp0, pool, ptag)

            def v1(kc):
                return lambda: emit_v_convoy(1, kc, xts1_box[0], vp1,
                                             psum_f, "f")

            def load_x1():
                xts1_box[0], xts1_box[1] = load_x(1)

            KOF = NCC  # k output-chunk offset
            fill0 = {
                (0, 1): [v0(0, psum_o, "o")],
                (0, 2): [v0(1, psum_f, "f"), load_x1],
                (0, 3): [v0(2, psum_o, "o")],
                (0, 4): [v0(3, psum_f, "f"), load_pw],
                (0, 5): [v0(4, psum_o, "o")],
                (0, 6): [v0(5, psum_f, "f")],
                (1, 1): [v0(6, psum_f, "f")],
                (1, 3): [qk0(KOF + 1)],          # k1 (needed h=2)
                (1, 6): [qk0(2)],                # q2 (h=4)
                (2, 2): [qk0(KOF + 2)],          # k2
                (2, 5): [qk0(3)],                # q3 (h=6)
                (3, 2): [qk0(KOF + 3)],          # k3
                (3, 5): [qk0(4)],                # q4 (h=8)
                (4, 2): [qk0(KOF + 4)],          # k4
                (4, 5): [qk0(5)],                # q5 (h=10)
                (5, 2): [qk0(KOF + 5)],          # k5
                (5, 5): [qk1(0)],
                (6, 2): [qk1(KOF + 0)],
                (6, 5): [v1(0)],
                (7, 2): [v1(1)],
                (8, 2): [v1(2)],
                (9, 2): [v1(3)],
                (10, 2): [v1(4)],
                (10, 5): [v1(5)],
                (11, 2): [qk1(1)],               # needed attn1 h=2
                (11, 5): [qk1(KOF + 1)],
            }

            def proj0(tt):
                return lambda: emit_proj_convoy(0, tt, oT0, psum_f, "f")

            def n0(j):
                return lambda: emit_norm_pair(0, j, oT0)

            def n1(j):
                return lambda: emit_norm_pair(1, j, oT1)

            fill1 = {
                (0, 1): [v1(6)],
                (0, 2): [n0(0)],
                (0, 3): [qk1(2)],                # needed h=4
                (0, 4): [n0(1)],
                (0, 6): [qk1(KOF + 2)],
                (1, 1): [n0(2)],
                (1, 2): [qk1(3)],                # h=6
                (1, 4): [n0(3)],
                (1, 5): [qk1(KOF + 3)],
                (2, 1): [n0(4)],
                (2, 2): [qk1(4)],                # h=8
                (2, 4): [n0(5)],
                (2, 5): [qk1(KOF + 4)],
                (3, 2): [qk1(5)],                # h=10
                (3, 5): [qk1(KOF + 5)],
                (4, 2): [proj0(0)],
                (5, 1): [n1(0)],
                (5, 2): [proj0(1)],
                (6, 2): [proj0(2)],
                (7, 1): [n1(1)],
                (7, 2): [proj0(3)],
                (8, 2): [proj0(4)],
                (9, 1): [n1(2)],
                (9, 2): [proj0(5)],
                (10, 2): [proj0(6)],
                (11, 1): [n1(3)],
                (H, 0): [lambda: emit_norm_pair(1, 4, oT1, deng=nc.scalar)],
            }

            emit_attention(0, qT0, kT0, vp0, oT0, fill0)
            emit_attention(1, qT1, kT1, vp1, oT1, fill1)

            # tail: last normalize pair's DMA chain hides under split proj
            # accumulation — contraction chunks 0-4 (pairs already normalized)
            # run across 4 open psum slots while pair 5's 1/d lands; chunk 5
            # joins in part2.
            emit_norm_pair(1, 5, oT1, deng=nc.scalar)
            tail_ps = {}
            tail_pool = [(psum_s, "s"), (psum_f, "f"),
                         (psum_o, "o"), (psum_s, "s")]
            for tt in range(4):
                pool, ptag = tail_pool[tt]
                tail_ps[tt] = emit_proj_part1(1, tt, oT1, pool, ptag, NCC - 1)
            for tt in range(4):
                emit_proj_part2(1, tt, oT1, tail_ps[tt], NCC - 1)
            for i, tt in enumerate(range(4, NKC)):
                pool, ptag = [(psum_f, "f"), (psum_o, "o"),
                              (psum_s, "s")][i % 3]
                emit_proj_convoy(1, tt, oT1, pool, ptag)

    nc.compile()
    return nc


def _relative_position_index():
    coords = np.stack(np.meshgrid(np.arange(WX), np.arange(WY), indexing="ij"))
    cf = coords.reshape(2, -1)
    rel = cf[:, :, None] - cf[:, None, :]
    rel = rel.transpose(1, 2, 0).astype(np.int64)
    rel[:, :, 0] += WX - 1
    rel[:, :, 1] += WY - 1
    rel[:, :, 0] *= 2 * WY - 1
    return rel.sum(-1)  # [L, L]


def _host_prep(x, qkv_w, proj_w, proj_b, rel_table, g2l, g2g):
    x = np.asarray(x, np.float32)
    qkv_w = np.asarray(qkv_w, np.float32)
    proj_w = np.asarray(proj_w, np.float32)
    proj_b = np.asarray(proj_b, np.float32)
    rel_table = np.asarray(rel_table, np.float32)
    g2l = np.asarray(g2l, np.float32)
    g2g = np.asarray(g2g, np.float32)

    bf16 = ml_dtypes.bfloat16
    f8 = ml_dtypes.float8_e4m3fn
    # x and q/k weights ship as fp8e4 in the [.., 128, 2, *] DoubleRow
    # layout: contraction chunk p covers C-rows [256p, 256p+256), subtile
    # s = rows [256p+128s, +128). Weight columns pre-scaled into fp8's
    # normal range; the psum evacuation rescales by 1/QS, 1/KS.
    xT = x.transpose(0, 2, 1)                                      # [B, C, N]
    xT16 = np.ascontiguousarray(xT).astype(bf16)
    x8 = np.clip(xT, -240, 240).astype(f8)
    x8 = np.ascontiguousarray(
        x8.reshape(B, 3, 2, 128, N).transpose(0, 1, 3, 2, 4))     # [B,3,128,2,N]
    qk_wT = qkv_w[:2 * C].T.copy()                                 # [C, 2C]
    qk_wT[:, :C] *= SCALE * QS
    qk_wT[:, C:] *= KS
    w8 = np.clip(qk_wT, -240, 240).astype(f8)
    # [C, 2C] -> [ccp 3, block 2, 128, 2, 768]
    w8 = np.ascontiguousarray(
        w8.reshape(3, 2, 128, 2, C).transpose(0, 3, 2, 1, 4))
    v_wT = np.ascontiguousarray(qkv_w[2 * C:].T).astype(bf16)      # [C, C]
    proj_wT = np.ascontiguousarray(proj_w.T).astype(bf16)          # [C, C]
    pb = proj_b.reshape(1, C).astype(bf16)

    # expB[h, k, q] = exp(bias[h, q, k]); exp applied at table granularity,
    # then expanded by the constant-index relative-position gather.
    ridx = _relative_position_index()
    et = np.exp(rel_table)                                         # [3025, H]
    eg2l = np.exp(g2l)                                             # [2, H, 1]
    eg2g = np.exp(g2g)                                             # [H, 1, 1]
    expB = np.empty((H, N, N), np.float32)
    expB[:, 1:, 1:] = et[ridx].transpose(2, 1, 0)                  # [H, k, q]
    expB[:, 0, 0] = eg2g[:, 0, 0]
    expB[:, 1:, 0] = eg2l[0][:, 0][None, :].T                      # global query
    expB[:, 0, 1:] = eg2l[1][:, 0][:, None]                        # global key
    expB16 = expB.astype(bf16)

    in_maps = []
    for i in range(N_CORES):
        in_maps.append({
            "xT": xT16[i * B_LOC:(i + 1) * B_LOC],
            "x8": x8[i * B_LOC:(i + 1) * B_LOC],
            "qk_w8": w8,
            "v_wT": v_wT,
            "proj_wT": proj_wT,
            "proj_b": pb,
            "expB": expB16,
        })
    return in_maps


_NC = None


def get_nc():
    global _NC
    if _NC is None:
        _NC = build_nc()
    return _NC


def kernel(x, qkv_w, proj_w, proj_b, rel_table, g2l, g2g):
    in_maps = _host_prep(x, qkv_w, proj_w, proj_b, rel_table, g2l, g2g)
    nc = get_nc()
    res = run_bass_kernel_spmd(nc, in_maps, core_ids=list(range(N_CORES)))
    out = np.concatenate([res.results[i]["out"] for i in range(N_CORES)], axis=0)
    return out.astype(np.float32)
